# revision 14
# baseline (speedup 1.0000x reference)
"""Trainium2 Bass kernel for nn_MultiHeadAttention_9491877724818.

Math (per batch b, head h), reformulated from the reference:
    q = Wq_h @ x_b + bq          (128, T)
    k = Wk_h @ x_b + bk          (128, T)
    eT[s,t] = (k.T @ q)[s,t]     == energy[t,s]; softmax over s (partition dim)
    expET = exp(eT)              (no max subtraction: |logit| <= ~70, fp32-safe)
    Z[t] = sum_s expET[s,t]      (PE ones-matmul -> broadcast across partitions)
Key algebraic folding: heads only enter the output through W1 (DFC1=128 rows),
so the huge Wv (C x C) conv and o = v @ attn (each 2.1 GF/bh) collapse into
128-channel products:
    vW1T[s,j]  = (x_b.T @ (W1 @ Wv_h).T)[s,j]          (T, 128)
    oW1raw[j,t]= sum_s vW1T[s,j] expET[s,t]            (128, T)
    fc1[j,t]   = relu(gamma_h * oW1raw[j,t]/Z[t] + xW1[b][j,t] + b1eff_h[j])
        where xW1 = W1 @ x_b, b1eff = b1 + gamma_h * (W1 @ bv_h)
        (softmax rows sum to 1 => v-bias passes through as a constant)
    out2[d,t]  = relu(W2 @ fc1 + b2)
    final[b, 8*d + h, t] = out2[d,t] + x[b, 8*d + h, t]

Sharding: data parallel - core i computes batch b=i entirely (all 8 heads).

Dtypes: two matmul families. The logit path (QK convs + k.T@q) needs accuracy
because exp() amplifies absolute logit error; the post-softmax path is plain
linear algebra where bf16 (~0.2% rel) is fine. float32r = fp32 storage with
reduced-precision PE multiply (~2 cyc/row measured); bf16 = 1 cyc/row.
"""

import numpy as np

B, C, T, H, P = 8, 1024, 1024, 8, 128
CT = C // P      # 8 contraction k-tiles over channels
ST = T // P      # 8 s-tiles (softmax/partition dim)
NT = 2           # t-chunks per row
TCW = T // NT    # 512 = matmul moving free dim

# (logit_dt, mlp_dt)
CONFIG = ("float16", "bfloat16")

_module_cache = {}


def _build_module(cfg=CONFIG):
    logit_name, mlp_name = cfg
    from contextlib import ExitStack

    import concourse.bacc as bacc
    import concourse.bass as bass
    import concourse.mybir as mybir
    import concourse.tile as tile

    f32 = mybir.dt.float32
    ldt = getattr(mybir.dt, logit_name)
    mdt = getattr(mybir.dt, mlp_name)
    AF = mybir.ActivationFunctionType
    ALU = mybir.AluOpType

    def is4(dt):
        return mybir.dt.size(dt) == 4

    nc = bacc.Bacc(trn_type="TRN2", name="mha_dp")

    # f32 x always present (residual source; logit source when ldt is 4-byte)
    x_d = nc.dram_tensor("x", (C, T), f32, kind="ExternalInput")
    # narrow copies of x per 2-byte matmul family in use
    xnarrow = {}
    for dt_ in {d for d in (ldt, mdt) if mybir.dt.size(d) == 2}:
        xnarrow[dt_] = nc.dram_tensor(f"x_{dt_.name}", (C, T), dt_, kind="ExternalInput")

    wqk_d = nc.dram_tensor("wqk", (H, P, CT, 256), f32 if is4(ldt) else ldt, kind="ExternalInput")
    w1wv_d = nc.dram_tensor("w1wv", (P, CT, H * P), f32 if is4(mdt) else mdt, kind="ExternalInput")
    w1t_d = nc.dram_tensor("w1t", (P, CT, P), f32 if is4(mdt) else mdt, kind="ExternalInput")
    w2t_d = nc.dram_tensor("w2t", (P, P), f32 if is4(mdt) else mdt, kind="ExternalInput")
    ones_d = nc.dram_tensor("ones", (P, P), f32 if is4(mdt) else mdt, kind="ExternalInput")
    bqk_d = nc.dram_tensor("bqk", (H, 2, P), f32, kind="ExternalInput")
    b1e_d = nc.dram_tensor("b1e", (H, P), f32, kind="ExternalInput")
    b2_d = nc.dram_tensor("b2", (P,), f32, kind="ExternalInput")
    gam_d = nc.dram_tensor("gam", (H,), f32, kind="ExternalInput")
    out_d = nc.dram_tensor("out", (C, T), f32, kind="ExternalOutput")

    def load(sb_ap, dram_ap, dt):
        # 4-byte matmul dtypes view the f32 bytes in place; 2-byte comes
        # from host-cast arrays whose DRAM dtype already matches.
        if is4(dt):
            nc.sync.dma_start(out=sb_ap, in_=dram_ap.bitcast(dt))
        else:
            nc.sync.dma_start(out=sb_ap, in_=dram_ap)

    def mm(ps, lhsT, rhs, start, stop):
        nc.tensor.matmul(ps, lhsT, rhs, start=start, stop=stop)

    with tile.TileContext(nc) as tc, ExitStack() as ctx:
        consts = ctx.enter_context(tc.tile_pool(name="consts", bufs=1))
        psA = ctx.enter_context(tc.tile_pool(name="psA", bufs=3, space="PSUM"))
        psB = ctx.enter_context(tc.tile_pool(name="psB", bufs=1, space="PSUM"))

        wqkp = ctx.enter_context(tc.tile_pool(name="wqkp", bufs=2))
        qkp = ctx.enter_context(tc.tile_pool(name="qkp", bufs=2))
        expp = ctx.enter_context(tc.tile_pool(name="expp", bufs=3))
        hbuf = ctx.enter_context(tc.tile_pool(name="hbuf", bufs=2))
        outp = ctx.enter_context(tc.tile_pool(name="outp", bufs=2))

        # ---------------- constants + phase A (per-batch, head-independent)
        # DMA priority order: tensors gating the first matmuls come first.
        w1t_sb = consts.tile([P, CT, P], mdt, name="w1t_sb")
        load(w1t_sb, w1t_d[:], mdt)
        if mlp_name == logit_name:
            xm_sb = consts.tile([P, CT, T], mdt, name="xm_sb")
            for ci in range(CT):
                src = x_d if is4(mdt) else xnarrow[mdt]
                load(xm_sb[:, ci, :], src[ci * P : (ci + 1) * P, :], mdt)
            xl_sb = xm_sb
        else:
            xm_sb = consts.tile([P, CT, T], mdt, name="xm_sb")
            for ci in range(CT):
                src = x_d if is4(mdt) else xnarrow[mdt]
                load(xm_sb[:, ci, :], src[ci * P : (ci + 1) * P, :], mdt)
            xl_sb = consts.tile([P, CT, T], ldt, name="xl_sb")
            for ci in range(CT):
                src = x_d if is4(ldt) else xnarrow[ldt]
                load(xl_sb[:, ci, :], src[ci * P : (ci + 1) * P, :], ldt)
        w1wv_sb = consts.tile([P, CT, H * P], mdt, name="w1wv_sb")
        for ci in range(CT):
            load(w1wv_sb[:, ci, :], w1wv_d[:, ci, :], mdt)
        w2t_sb = consts.tile([P, P], mdt, name="w2t_sb")
        load(w2t_sb, w2t_d[:], mdt)
        ones_sb = consts.tile([P, P], mdt, name="ones_sb")
        load(ones_sb, ones_d[:], mdt)
        b2_sb = consts.tile([P, 1], f32, name="b2_sb")
        nc.sync.dma_start(out=b2_sb, in_=b2_d[:])

        xw1_sb = consts.tile([P, T], f32, name="xw1_sb")
        vw1t_sb = consts.tile([P, ST, H * P], mdt, name="vw1t_sb")

        # ---------------- per-head pipeline, software-pipelined emission.
        # PE executes its queue in order, so a chunk's consumers must be
        # emitted AFTER the next chunk's independent matmuls or the PE idles
        # waiting on the ACT/DVE chain. Stages per chunk c=(h,t2):
        #   S1(c): eT matmuls + exp          (emitted at c)
        #   S2(c): oW1 + Z matmuls            (emitted at c+1)
        #   S3(c): DVE normalize chain -> fc1 (emitted at c+1)
        #   S4(c): FC2 matmul                 (emitted at c+2)
        #   S5(c): out relu + residual add    (emitted at c+2)
        head_state = {}
        chunk_state = {}

        def emit_head_setup(h):
            wqk_sb = wqkp.tile([P, CT, 256], ldt, name="wqk_sb", tag="wqk")
            load(wqk_sb, wqk_d[h], ldt)
            bq_sb = wqkp.tile([P, 1], f32, name="bq_sb", tag="bq")
            nc.sync.dma_start(out=bq_sb, in_=bqk_d[h, 0, :])
            bk_sb = wqkp.tile([P, 1], f32, name="bk_sb", tag="bk")
            nc.sync.dma_start(out=bk_sb, in_=bqk_d[h, 1, :])
            b1e_sb = wqkp.tile([P, 1], f32, name="b1e_sb", tag="b1e")
            nc.sync.dma_start(out=b1e_sb, in_=b1e_d[h, :])
            gam_sb = wqkp.tile([P, 1], f32, name="gam_sb", tag="gam")
            gam_ap = gam_d[h : h + 1]
            nc.gpsimd.dma_start(
                out=gam_sb,
                in_=bass.AP(tensor=gam_ap.tensor, offset=gam_ap.offset, ap=[[0, P], [1, 1]]),
            )
            # residual rows h, h+8, ..., h+8*127 of x
            xres_sb = wqkp.tile([P, T], f32, name="xres_sb", tag="xres")
            x_all = x_d[:]
            nc.sync.dma_start(
                out=xres_sb,
                in_=bass.AP(tensor=x_all.tensor, offset=h * T, ap=[[H * T, P], [1, T]]),
            )
            q_sb = qkp.tile([P, T], ldt, name="q_sb", tag="q")
            k_sb = qkp.tile([P, T], ldt, name="k_sb", tag="k")
            out_sb = outp.tile([P, T], f32, name="out_sb", tag="ob")
            # K then Q, both chunks each; bias-adds hide under later MM groups
            for t2 in range(NT):
                tsl = slice(t2 * TCW, (t2 + 1) * TCW)
                ps_k = psB.tile([P, TCW], f32, name="ps_k", tag="qk", bufs=2)
                for ci in range(CT):
                    mm(ps_k, wqk_sb[:, ci, P : 2 * P], xl_sb[:, ci, tsl], ci == 0, ci == CT - 1)
                nc.vector.tensor_scalar_add(out=k_sb[:, tsl], in0=ps_k, scalar1=bk_sb)
            for t2 in range(NT):
                tsl = slice(t2 * TCW, (t2 + 1) * TCW)
                ps_q = psB.tile([P, TCW], f32, name="ps_q", tag="qk", bufs=2)
                for ci in range(CT):
                    mm(ps_q, wqk_sb[:, ci, 0:P], xl_sb[:, ci, tsl], ci == 0, ci == CT - 1)
                nc.vector.tensor_scalar_add(out=q_sb[:, tsl], in0=ps_q, scalar1=bq_sb)
            head_state[h] = dict(
                q=q_sb, k=k_sb, gam=gam_sb, b1e=b1e_sb, xres=xres_sb, out=out_sb
            )

        def emit_s1(c):
            h, t2 = c
            hs = head_state[h]
            tsl = slice(t2 * TCW, (t2 + 1) * TCW)
            et_sb = expp.tile([P, ST, TCW], mdt, name="et_sb", tag="exp")
            for si in range(ST):
                ps_e = psA.tile([P, TCW], f32, name="ps_e", tag="acc")
                mm(ps_e, hs["k"][:, si * P : (si + 1) * P], hs["q"][:, tsl], True, True)
                nc.scalar.activation(out=et_sb[:, si, :], in_=ps_e, func=AF.Exp)
            chunk_state[c] = dict(et=et_sb)

        def emit_s2_s3(c):
            h, t2 = c
            hs = head_state[h]
            cs = chunk_state[c]
            tsl = slice(t2 * TCW, (t2 + 1) * TCW)
            et_sb = cs["et"]
            ps_o = psB.tile([P, TCW], f32, name="ps_o", tag="oo")
            for si in range(ST):
                mm(
                    ps_o,
                    vw1t_sb[:, si, h * P : (h + 1) * P],
                    et_sb[:, si, :],
                    si == 0,
                    si == ST - 1,
                )
            ps_z = psB.tile([P, TCW], f32, name="ps_z", tag="zz")
            for si in range(ST):
                mm(ps_z, ones_sb, et_sb[:, si, :], si == 0, si == ST - 1)
            # fc1 = relu(gamma * oW1/Z + xW1 + b1eff), all on DVE
            izg = hbuf.tile([P, TCW], f32, name="izg", tag="izg")
            nc.vector.reciprocal_approx_fast(out=izg, in_=ps_z)
            t1 = hbuf.tile([P, TCW], f32, name="t1", tag="t1")
            nc.vector.scalar_tensor_tensor(
                out=t1, in0=ps_o, scalar=hs["gam"], in1=izg, op0=ALU.mult, op1=ALU.mult
            )
            t2t = hbuf.tile([P, TCW], f32, name="t2t", tag="t2t")
            nc.vector.scalar_tensor_tensor(
                out=t2t, in0=t1, scalar=hs["b1e"], in1=xw1_sb[:, tsl], op0=ALU.add, op1=ALU.add
            )
            fc1 = hbuf.tile([P, TCW], mdt, name="fc1", tag="fc1")
            nc.vector.tensor_scalar_max(out=fc1, in0=t2t, scalar1=0.0)
            cs["fc1"] = fc1

        def emit_s4_s5(c):
            h, t2 = c
            hs = head_state[h]
            cs = chunk_state[c]
            tsl = slice(t2 * TCW, (t2 + 1) * TCW)
            ps_f = psB.tile([P, TCW], f32, name="ps_f", tag="ff")
            mm(ps_f, w2t_sb[:], cs["fc1"], True, True)
            ot = hbuf.tile([P, TCW], f32, name="ot", tag="ot")
            nc.scalar.activation(out=ot, in_=ps_f, func=AF.Relu, bias=b2_sb)
            nc.vector.tensor_add(hs["out"][:, tsl], ot, hs["xres"][:, tsl])
            out_all = out_d[:]
            nc.sync.dma_start(
                out=bass.AP(
                    tensor=out_all.tensor,
                    offset=h * T + t2 * TCW,
                    ap=[[H * T, P], [1, TCW]],
                ),
                in_=hs["out"][:, tsl],
            )

        # ---- phase A compute: xW1 first (smallest DMA deps), head-0 QK
        # next (overlaps w1wv DMA), then vW1T.
        for t2 in range(NT):
            tsl = slice(t2 * TCW, (t2 + 1) * TCW)
            ps_x = psA.tile([P, TCW], f32, name="ps_x", tag="acc")
            for ci in range(CT):
                mm(ps_x, w1t_sb[:, ci, :], xm_sb[:, ci, tsl], ci == 0, ci == CT - 1)
            nc.vector.tensor_copy(out=xw1_sb[:, tsl], in_=ps_x)

        emit_head_setup(0)
        emit_s1((0, 0))

        # vW1T = x.T @ W1WvT for all heads  (s on partitions, h*128+j free)
        for si in range(ST):
            for jh in range(2):
                jsl = slice(jh * 512, (jh + 1) * 512)
                ps_v = psA.tile([P, TCW], f32, name="ps_v", tag="acc")
                for ci in range(CT):
                    mm(
                        ps_v,
                        xm_sb[:, ci, si * P : (si + 1) * P],
                        w1wv_sb[:, ci, jsl],
                        ci == 0,
                        ci == CT - 1,
                    )
                nc.vector.tensor_copy(out=vw1t_sb[:, si, jsl], in_=ps_v)

        chunks = [(h, t2) for h in range(H) for t2 in range(NT)]
        for i, c in enumerate(chunks):
            if c[1] == 0 and c[0] != 0:
                emit_head_setup(c[0])
            if c != (0, 0):
                emit_s1(c)
            if i >= 1:
                emit_s2_s3(chunks[i - 1])
            if i >= 2:
                emit_s4_s5(chunks[i - 2])
        emit_s2_s3(chunks[-1])
        emit_s4_s5(chunks[-2])
        emit_s4_s5(chunks[-1])

    nc.compile()
    return nc


def _prepare_inputs(inputs, cfg=CONFIG):
    import ml_dtypes

    logit_name, mlp_name = cfg
    l4 = logit_name in ("float32", "float32r")
    m4 = mlp_name in ("float32", "float32r")
    np_dt = {"bfloat16": ml_dtypes.bfloat16, "float16": np.float16}

    x = np.ascontiguousarray(np.asarray(inputs["x"], dtype=np.float32))
    Wq = np.asarray(inputs["Wq"], dtype=np.float32)
    bq = np.asarray(inputs["bq"], dtype=np.float32)
    Wk = np.asarray(inputs["Wk"], dtype=np.float32)
    bk = np.asarray(inputs["bk"], dtype=np.float32)
    Wv = np.asarray(inputs["Wv"], dtype=np.float32)
    bv = np.asarray(inputs["bv"], dtype=np.float32)
    gamma = np.asarray(inputs["gamma"], dtype=np.float32)
    W1 = np.asarray(inputs["W1"], dtype=np.float32)
    b1 = np.asarray(inputs["b1"], dtype=np.float32)
    W2 = np.asarray(inputs["W2"], dtype=np.float32)
    b2 = np.asarray(inputs["b2"], dtype=np.float32)

    # wqk[h, cp, ci, 0:128] = Wq[h].T[ci*128+cp, :]; 128:256 for Wk
    wqk = np.empty((H, P, CT, 256), dtype=np.float32)
    for h in range(H):
        wqk[h, :, :, 0:P] = Wq[h].T.reshape(CT, P, P).transpose(1, 0, 2)
        wqk[h, :, :, P : 2 * P] = Wk[h].T.reshape(CT, P, P).transpose(1, 0, 2)

    # w1wv[cp, ci, h*128+j] = (W1 @ Wv[h]).T[ci*128+cp, j]
    w1wvT = np.concatenate([(W1 @ Wv[h]).T for h in range(H)], axis=1)  # (C, H*128)
    w1wv = np.ascontiguousarray(w1wvT.reshape(CT, P, H * P).transpose(1, 0, 2))

    w1t = np.ascontiguousarray(W1.T.reshape(CT, P, P).transpose(1, 0, 2))
    w2t = np.ascontiguousarray(W2.T)

    bqk = np.stack([bq, bk], axis=1)  # (H, 2, P)
    b1v = bv @ W1.T  # (H, P): b1v[h] = W1 @ bv[h]
    b1e = b1[None, :] + gamma[:, None] * b1v  # (H, P)

    def fam(arr, four_byte, name):
        arr = np.ascontiguousarray(arr, dtype=np.float32)
        return arr if four_byte else np.ascontiguousarray(arr.astype(np_dt[name]))

    shared = {
        "wqk": fam(wqk, l4, logit_name),
        "w1wv": fam(w1wv, m4, mlp_name),
        "w1t": fam(w1t, m4, mlp_name),
        "w2t": fam(w2t, m4, mlp_name),
        "ones": fam(np.ones((P, P), dtype=np.float32), m4, mlp_name),
        "bqk": np.ascontiguousarray(bqk),
        "b1e": np.ascontiguousarray(b1e),
        "b2": np.ascontiguousarray(b2),
        "gam": np.ascontiguousarray(gamma),
    }
    narrow_names = {n for n, four in ((logit_name, l4), (mlp_name, m4)) if not four}
    in_maps = []
    for b in range(B):
        m = dict(shared)
        m["x"] = np.ascontiguousarray(x[b])
        for n in narrow_names:
            m[f"x_{n}"] = np.ascontiguousarray(x[b].astype(np_dt[n]))
        in_maps.append(m)
    return in_maps


def kernel(**inputs):
    from concourse.bass_utils import run_bass_kernel_spmd

    if CONFIG not in _module_cache:
        _module_cache[CONFIG] = _build_module(CONFIG)
    nc = _module_cache[CONFIG]

    in_maps = _prepare_inputs(inputs, CONFIG)
    res = run_bass_kernel_spmd(nc, in_maps, core_ids=list(range(B)))
    out = np.stack([res.results[b]["out"] for b in range(B)], axis=0)
    return out.astype(np.float32)


# revision 15
# speedup vs baseline: 1.0724x; 1.0724x over previous
"""Trainium2 Bass kernel for nn_MultiHeadAttention_9491877724818.

Math (per batch b, head h), reformulated from the reference:
    q = Wq_h @ x_b + bq          (128, T)
    k = Wk_h @ x_b + bk          (128, T)
    eT[s,t] = (k.T @ q)[s,t]     == energy[t,s]; softmax over s (partition dim)
    expET = exp(eT)              (no max subtraction: |logit| <= ~70, fp32-safe)
    Z[t] = sum_s expET[s,t]      (PE ones-matmul -> broadcast across partitions)
Key algebraic folding: heads only enter the output through W1 (DFC1=128 rows),
so the huge Wv (C x C) conv and o = v @ attn (each 2.1 GF/bh) collapse into
128-channel products:
    vW1T[s,j]  = (x_b.T @ (W1 @ Wv_h).T)[s,j]          (T, 128)
    oW1raw[j,t]= sum_s vW1T[s,j] expET[s,t]            (128, T)
    fc1[j,t]   = relu(gamma_h * oW1raw[j,t]/Z[t] + xW1[b][j,t] + b1eff_h[j])
        where xW1 = W1 @ x_b, b1eff = b1 + gamma_h * (W1 @ bv_h)
        (softmax rows sum to 1 => v-bias passes through as a constant)
    out2[d,t]  = relu(W2 @ fc1 + b2)
    final[b, 8*d + h, t] = out2[d,t] + x[b, 8*d + h, t]

Sharding: data parallel - core i computes batch b=i entirely (all 8 heads).

Dtypes: two matmul families. The logit path (QK convs + k.T@q) needs accuracy
because exp() amplifies absolute logit error; the post-softmax path is plain
linear algebra where bf16 (~0.2% rel) is fine. float32r = fp32 storage with
reduced-precision PE multiply (~2 cyc/row measured); bf16 = 1 cyc/row.
"""

import numpy as np

B, C, T, H, P = 8, 1024, 1024, 8, 128
CT = C // P      # 8 contraction k-tiles over channels
ST = T // P      # 8 s-tiles (softmax/partition dim)
NT = 2           # t-chunks per row
TCW = T // NT    # 512 = matmul moving free dim

# (logit_dt, mlp_dt)
CONFIG = ("float16", "bfloat16")

_module_cache = {}


def _build_module(cfg=CONFIG):
    logit_name, mlp_name = cfg
    from contextlib import ExitStack

    import concourse.bacc as bacc
    import concourse.bass as bass
    import concourse.mybir as mybir
    import concourse.tile as tile

    f32 = mybir.dt.float32
    ldt = getattr(mybir.dt, logit_name)
    mdt = getattr(mybir.dt, mlp_name)
    AF = mybir.ActivationFunctionType
    ALU = mybir.AluOpType

    def is4(dt):
        return mybir.dt.size(dt) == 4

    nc = bacc.Bacc(trn_type="TRN2", name="mha_dp")

    # f32 x always present (residual source; logit source when ldt is 4-byte)
    x_d = nc.dram_tensor("x", (C, T), f32, kind="ExternalInput")
    # narrow copies of x per 2-byte matmul family in use
    xnarrow = {}
    for dt_ in {d for d in (ldt, mdt) if mybir.dt.size(d) == 2}:
        xnarrow[dt_] = nc.dram_tensor(f"x_{dt_.name}", (C, T), dt_, kind="ExternalInput")

    wqk_d = nc.dram_tensor("wqk", (H, P, CT, 256), f32 if is4(ldt) else ldt, kind="ExternalInput")
    w1wv_d = nc.dram_tensor("w1wv", (P, CT, H * P), f32 if is4(mdt) else mdt, kind="ExternalInput")
    w1t_d = nc.dram_tensor("w1t", (P, CT, P), f32 if is4(mdt) else mdt, kind="ExternalInput")
    w2t_d = nc.dram_tensor("w2t", (P, P), f32 if is4(mdt) else mdt, kind="ExternalInput")
    ones_d = nc.dram_tensor("ones", (P, P), f32 if is4(mdt) else mdt, kind="ExternalInput")
    bqk_d = nc.dram_tensor("bqk", (H, 2, P), f32, kind="ExternalInput")
    b1e_d = nc.dram_tensor("b1e", (H, P), f32, kind="ExternalInput")
    b2_d = nc.dram_tensor("b2", (P,), f32, kind="ExternalInput")
    gam_d = nc.dram_tensor("gam", (H,), f32, kind="ExternalInput")
    out_d = nc.dram_tensor("out", (C, T), f32, kind="ExternalOutput")

    def load(sb_ap, dram_ap, dt):
        # 4-byte matmul dtypes view the f32 bytes in place; 2-byte comes
        # from host-cast arrays whose DRAM dtype already matches.
        if is4(dt):
            nc.sync.dma_start(out=sb_ap, in_=dram_ap.bitcast(dt))
        else:
            nc.sync.dma_start(out=sb_ap, in_=dram_ap)

    def mm(ps, lhsT, rhs, start, stop):
        nc.tensor.matmul(ps, lhsT, rhs, start=start, stop=stop)

    with tile.TileContext(nc) as tc, ExitStack() as ctx:
        consts = ctx.enter_context(tc.tile_pool(name="consts", bufs=1))
        psA = ctx.enter_context(tc.tile_pool(name="psA", bufs=3, space="PSUM"))
        psB = ctx.enter_context(tc.tile_pool(name="psB", bufs=1, space="PSUM"))

        wqkp = ctx.enter_context(tc.tile_pool(name="wqkp", bufs=2))
        qkp = ctx.enter_context(tc.tile_pool(name="qkp", bufs=2))
        expp = ctx.enter_context(tc.tile_pool(name="expp", bufs=3))
        hbuf = ctx.enter_context(tc.tile_pool(name="hbuf", bufs=2))
        outp = ctx.enter_context(tc.tile_pool(name="outp", bufs=2))

        # ---------------- constants + phase A (per-batch, head-independent)
        # DMA priority order: tensors gating the first matmuls come first.
        w1t_sb = consts.tile([P, CT, P], mdt, name="w1t_sb")
        load(w1t_sb, w1t_d[:], mdt)
        if mlp_name == logit_name:
            xm_sb = consts.tile([P, CT, T], mdt, name="xm_sb")
            for ci in range(CT):
                src = x_d if is4(mdt) else xnarrow[mdt]
                load(xm_sb[:, ci, :], src[ci * P : (ci + 1) * P, :], mdt)
            xl_sb = xm_sb
        else:
            xm_sb = consts.tile([P, CT, T], mdt, name="xm_sb")
            for ci in range(CT):
                src = x_d if is4(mdt) else xnarrow[mdt]
                load(xm_sb[:, ci, :], src[ci * P : (ci + 1) * P, :], mdt)
        w1wv_sb = consts.tile([P, CT, H * P], mdt, name="w1wv_sb")
        for ci in range(CT):
            load(w1wv_sb[:, ci, :], w1wv_d[:, ci, :], mdt)
        if mlp_name != logit_name:
            xl_sb = consts.tile([P, CT, T], ldt, name="xl_sb")
            for ci in range(CT):
                src = x_d if is4(ldt) else xnarrow[ldt]
                load(xl_sb[:, ci, :], src[ci * P : (ci + 1) * P, :], ldt)
        w2t_sb = consts.tile([P, P], mdt, name="w2t_sb")
        load(w2t_sb, w2t_d[:], mdt)
        ones_sb = consts.tile([P, P], mdt, name="ones_sb")
        load(ones_sb, ones_d[:], mdt)
        b2_sb = consts.tile([P, 1], f32, name="b2_sb")
        nc.sync.dma_start(out=b2_sb, in_=b2_d[:])

        xw1_sb = consts.tile([P, T], f32, name="xw1_sb")
        vw1t_sb = consts.tile([P, ST, H * P], mdt, name="vw1t_sb")

        # ---------------- per-head pipeline, software-pipelined emission.
        # PE executes its queue in order, so a chunk's consumers must be
        # emitted AFTER the next chunk's independent matmuls or the PE idles
        # waiting on the ACT/DVE chain. Stages per chunk c=(h,t2):
        #   S1(c): eT matmuls + exp          (emitted at c)
        #   S2(c): oW1 + Z matmuls            (emitted at c+1)
        #   S3(c): DVE normalize chain -> fc1 (emitted at c+1)
        #   S4(c): FC2 matmul                 (emitted at c+2)
        #   S5(c): out relu + residual add    (emitted at c+2)
        head_state = {}
        chunk_state = {}

        def emit_head_setup(h):
            wqk_sb = wqkp.tile([P, CT, 256], ldt, name="wqk_sb", tag="wqk")
            load(wqk_sb, wqk_d[h], ldt)
            bq_sb = wqkp.tile([P, 1], f32, name="bq_sb", tag="bq")
            nc.sync.dma_start(out=bq_sb, in_=bqk_d[h, 0, :])
            bk_sb = wqkp.tile([P, 1], f32, name="bk_sb", tag="bk")
            nc.sync.dma_start(out=bk_sb, in_=bqk_d[h, 1, :])
            b1e_sb = wqkp.tile([P, 1], f32, name="b1e_sb", tag="b1e")
            nc.sync.dma_start(out=b1e_sb, in_=b1e_d[h, :])
            gam_sb = wqkp.tile([P, 1], f32, name="gam_sb", tag="gam")
            gam_ap = gam_d[h : h + 1]
            nc.gpsimd.dma_start(
                out=gam_sb,
                in_=bass.AP(tensor=gam_ap.tensor, offset=gam_ap.offset, ap=[[0, P], [1, 1]]),
            )
            # residual rows h, h+8, ..., h+8*127 of x
            xres_sb = wqkp.tile([P, T], f32, name="xres_sb", tag="xres")
            x_all = x_d[:]
            nc.sync.dma_start(
                out=xres_sb,
                in_=bass.AP(tensor=x_all.tensor, offset=h * T, ap=[[H * T, P], [1, T]]),
            )
            q_sb = qkp.tile([P, T], ldt, name="q_sb", tag="q")
            k_sb = qkp.tile([P, T], ldt, name="k_sb", tag="k")
            out_sb = outp.tile([P, T], f32, name="out_sb", tag="ob")
            # K then Q, both chunks each; bias-adds hide under later MM groups
            for t2 in range(NT):
                tsl = slice(t2 * TCW, (t2 + 1) * TCW)
                ps_k = psB.tile([P, TCW], f32, name="ps_k", tag="qk", bufs=2)
                for ci in range(CT):
                    mm(ps_k, wqk_sb[:, ci, P : 2 * P], xl_sb[:, ci, tsl], ci == 0, ci == CT - 1)
                nc.vector.tensor_scalar_add(out=k_sb[:, tsl], in0=ps_k, scalar1=bk_sb)
            for t2 in range(NT):
                tsl = slice(t2 * TCW, (t2 + 1) * TCW)
                ps_q = psB.tile([P, TCW], f32, name="ps_q", tag="qk", bufs=2)
                for ci in range(CT):
                    mm(ps_q, wqk_sb[:, ci, 0:P], xl_sb[:, ci, tsl], ci == 0, ci == CT - 1)
                nc.vector.tensor_scalar_add(out=q_sb[:, tsl], in0=ps_q, scalar1=bq_sb)
            head_state[h] = dict(
                q=q_sb, k=k_sb, gam=gam_sb, b1e=b1e_sb, xres=xres_sb, out=out_sb
            )

        def emit_s1(c):
            h, t2 = c
            hs = head_state[h]
            tsl = slice(t2 * TCW, (t2 + 1) * TCW)
            et_sb = expp.tile([P, ST, TCW], mdt, name="et_sb", tag="exp")
            for si in range(ST):
                ps_e = psA.tile([P, TCW], f32, name="ps_e", tag="acc")
                mm(ps_e, hs["k"][:, si * P : (si + 1) * P], hs["q"][:, tsl], True, True)
                nc.scalar.activation(out=et_sb[:, si, :], in_=ps_e, func=AF.Exp)
            chunk_state[c] = dict(et=et_sb)

        def emit_s2_s3(c):
            h, t2 = c
            hs = head_state[h]
            cs = chunk_state[c]
            tsl = slice(t2 * TCW, (t2 + 1) * TCW)
            et_sb = cs["et"]
            ps_o = psB.tile([P, TCW], f32, name="ps_o", tag="oo")
            for si in range(ST):
                mm(
                    ps_o,
                    vw1t_sb[:, si, h * P : (h + 1) * P],
                    et_sb[:, si, :],
                    si == 0,
                    si == ST - 1,
                )
            ps_z = psB.tile([P, TCW], f32, name="ps_z", tag="zz")
            for si in range(ST):
                mm(ps_z, ones_sb, et_sb[:, si, :], si == 0, si == ST - 1)
            # fc1 = relu(gamma * oW1/Z + xW1 + b1eff), all on DVE
            izg = hbuf.tile([P, TCW], f32, name="izg", tag="izg")
            nc.vector.reciprocal_approx_fast(out=izg, in_=ps_z)
            t1 = hbuf.tile([P, TCW], f32, name="t1", tag="t1")
            nc.vector.scalar_tensor_tensor(
                out=t1, in0=ps_o, scalar=hs["gam"], in1=izg, op0=ALU.mult, op1=ALU.mult
            )
            t2t = hbuf.tile([P, TCW], f32, name="t2t", tag="t2t")
            nc.vector.scalar_tensor_tensor(
                out=t2t, in0=t1, scalar=hs["b1e"], in1=xw1_sb[:, tsl], op0=ALU.add, op1=ALU.add
            )
            fc1 = hbuf.tile([P, TCW], mdt, name="fc1", tag="fc1")
            nc.vector.tensor_scalar_max(out=fc1, in0=t2t, scalar1=0.0)
            cs["fc1"] = fc1

        def emit_s4_s5(c):
            h, t2 = c
            hs = head_state[h]
            cs = chunk_state[c]
            tsl = slice(t2 * TCW, (t2 + 1) * TCW)
            ps_f = psB.tile([P, TCW], f32, name="ps_f", tag="ff")
            mm(ps_f, w2t_sb[:], cs["fc1"], True, True)
            ot = hbuf.tile([P, TCW], f32, name="ot", tag="ot")
            nc.scalar.activation(out=ot, in_=ps_f, func=AF.Relu, bias=b2_sb)
            nc.vector.tensor_add(hs["out"][:, tsl], ot, hs["xres"][:, tsl])
            out_all = out_d[:]
            nc.sync.dma_start(
                out=bass.AP(
                    tensor=out_all.tensor,
                    offset=h * T + t2 * TCW,
                    ap=[[H * T, P], [1, TCW]],
                ),
                in_=hs["out"][:, tsl],
            )

        # ---- phase A compute: xW1 first (smallest DMA deps), head-0 QK
        # next (overlaps w1wv DMA), then vW1T.
        for t2 in range(NT):
            tsl = slice(t2 * TCW, (t2 + 1) * TCW)
            ps_x = psA.tile([P, TCW], f32, name="ps_x", tag="acc")
            for ci in range(CT):
                mm(ps_x, w1t_sb[:, ci, :], xm_sb[:, ci, tsl], ci == 0, ci == CT - 1)
            nc.vector.tensor_copy(out=xw1_sb[:, tsl], in_=ps_x)

        # vW1T = x.T @ W1WvT for all heads  (s on partitions, h*128+j free)
        for si in range(ST):
            for jh in range(2):
                jsl = slice(jh * 512, (jh + 1) * 512)
                ps_v = psA.tile([P, TCW], f32, name="ps_v", tag="acc")
                for ci in range(CT):
                    mm(
                        ps_v,
                        xm_sb[:, ci, si * P : (si + 1) * P],
                        w1wv_sb[:, ci, jsl],
                        ci == 0,
                        ci == CT - 1,
                    )
                nc.vector.tensor_copy(out=vw1t_sb[:, si, jsl], in_=ps_v)

        chunks = [(h, t2) for h in range(H) for t2 in range(NT)]
        for i, c in enumerate(chunks):
            if c[1] == 0:
                emit_head_setup(c[0])
            emit_s1(c)
            if i >= 1:
                emit_s2_s3(chunks[i - 1])
            if i >= 2:
                emit_s4_s5(chunks[i - 2])
        emit_s2_s3(chunks[-1])
        emit_s4_s5(chunks[-2])
        emit_s4_s5(chunks[-1])

    nc.compile()
    return nc


def _prepare_inputs(inputs, cfg=CONFIG):
    import ml_dtypes

    logit_name, mlp_name = cfg
    l4 = logit_name in ("float32", "float32r")
    m4 = mlp_name in ("float32", "float32r")
    np_dt = {"bfloat16": ml_dtypes.bfloat16, "float16": np.float16}

    x = np.ascontiguousarray(np.asarray(inputs["x"], dtype=np.float32))
    Wq = np.asarray(inputs["Wq"], dtype=np.float32)
    bq = np.asarray(inputs["bq"], dtype=np.float32)
    Wk = np.asarray(inputs["Wk"], dtype=np.float32)
    bk = np.asarray(inputs["bk"], dtype=np.float32)
    Wv = np.asarray(inputs["Wv"], dtype=np.float32)
    bv = np.asarray(inputs["bv"], dtype=np.float32)
    gamma = np.asarray(inputs["gamma"], dtype=np.float32)
    W1 = np.asarray(inputs["W1"], dtype=np.float32)
    b1 = np.asarray(inputs["b1"], dtype=np.float32)
    W2 = np.asarray(inputs["W2"], dtype=np.float32)
    b2 = np.asarray(inputs["b2"], dtype=np.float32)

    # wqk[h, cp, ci, 0:128] = Wq[h].T[ci*128+cp, :]; 128:256 for Wk
    wqk = np.empty((H, P, CT, 256), dtype=np.float32)
    for h in range(H):
        wqk[h, :, :, 0:P] = Wq[h].T.reshape(CT, P, P).transpose(1, 0, 2)
        wqk[h, :, :, P : 2 * P] = Wk[h].T.reshape(CT, P, P).transpose(1, 0, 2)

    # w1wv[cp, ci, h*128+j] = (W1 @ Wv[h]).T[ci*128+cp, j]
    w1wvT = np.concatenate([(W1 @ Wv[h]).T for h in range(H)], axis=1)  # (C, H*128)
    w1wv = np.ascontiguousarray(w1wvT.reshape(CT, P, H * P).transpose(1, 0, 2))

    w1t = np.ascontiguousarray(W1.T.reshape(CT, P, P).transpose(1, 0, 2))
    w2t = np.ascontiguousarray(W2.T)

    bqk = np.stack([bq, bk], axis=1)  # (H, 2, P)
    b1v = bv @ W1.T  # (H, P): b1v[h] = W1 @ bv[h]
    b1e = b1[None, :] + gamma[:, None] * b1v  # (H, P)

    def fam(arr, four_byte, name):
        arr = np.ascontiguousarray(arr, dtype=np.float32)
        return arr if four_byte else np.ascontiguousarray(arr.astype(np_dt[name]))

    shared = {
        "wqk": fam(wqk, l4, logit_name),
        "w1wv": fam(w1wv, m4, mlp_name),
        "w1t": fam(w1t, m4, mlp_name),
        "w2t": fam(w2t, m4, mlp_name),
        "ones": fam(np.ones((P, P), dtype=np.float32), m4, mlp_name),
        "bqk": np.ascontiguousarray(bqk),
        "b1e": np.ascontiguousarray(b1e),
        "b2": np.ascontiguousarray(b2),
        "gam": np.ascontiguousarray(gamma),
    }
    narrow_names = {n for n, four in ((logit_name, l4), (mlp_name, m4)) if not four}
    in_maps = []
    for b in range(B):
        m = dict(shared)
        m["x"] = np.ascontiguousarray(x[b])
        for n in narrow_names:
            m[f"x_{n}"] = np.ascontiguousarray(x[b].astype(np_dt[n]))
        in_maps.append(m)
    return in_maps


def kernel(**inputs):
    from concourse.bass_utils import run_bass_kernel_spmd

    if CONFIG not in _module_cache:
        _module_cache[CONFIG] = _build_module(CONFIG)
    nc = _module_cache[CONFIG]

    in_maps = _prepare_inputs(inputs, CONFIG)
    res = run_bass_kernel_spmd(nc, in_maps, core_ids=list(range(B)))
    out = np.stack([res.results[b]["out"] for b in range(B)], axis=0)
    return out.astype(np.float32)


# revision 16
# speedup vs baseline: 1.0730x; 1.0006x over previous
"""Trainium2 Bass kernel for nn_MultiHeadAttention_9491877724818.

Math (per batch b, head h), reformulated from the reference:
    q = Wq_h @ x_b + bq          (128, T)
    k = Wk_h @ x_b + bk          (128, T)
    eT[s,t] = (k.T @ q)[s,t]     == energy[t,s]; softmax over s (partition dim)
    expET = exp(eT)              (no max subtraction: |logit| <= ~70, fp32-safe)
    Z[t] = sum_s expET[s,t]      (PE ones-matmul -> broadcast across partitions)
Key algebraic folding: heads only enter the output through W1 (DFC1=128 rows),
so the huge Wv (C x C) conv and o = v @ attn (each 2.1 GF/bh) collapse into
128-channel products:
    vW1T[s,j]  = (x_b.T @ (W1 @ Wv_h).T)[s,j]          (T, 128)
    oW1raw[j,t]= sum_s vW1T[s,j] expET[s,t]            (128, T)
    fc1[j,t]   = relu(gamma_h * oW1raw[j,t]/Z[t] + xW1[b][j,t] + b1eff_h[j])
        where xW1 = W1 @ x_b, b1eff = b1 + gamma_h * (W1 @ bv_h)
        (softmax rows sum to 1 => v-bias passes through as a constant)
    out2[d,t]  = relu(W2 @ fc1 + b2)
    final[b, 8*d + h, t] = out2[d,t] + x[b, 8*d + h, t]

Sharding: data parallel - core i computes batch b=i entirely (all 8 heads).

Dtypes: two matmul families. The logit path (QK convs + k.T@q) needs accuracy
because exp() amplifies absolute logit error; the post-softmax path is plain
linear algebra where bf16 (~0.2% rel) is fine. float32r = fp32 storage with
reduced-precision PE multiply (~2 cyc/row measured); bf16 = 1 cyc/row.
"""

import numpy as np

B, C, T, H, P = 8, 1024, 1024, 8, 128
CT = C // P      # 8 contraction k-tiles over channels
ST = T // P      # 8 s-tiles (softmax/partition dim)
NT = 2           # t-chunks per row
TCW = T // NT    # 512 = matmul moving free dim

# (logit_dt, mlp_dt)
CONFIG = ("float16", "bfloat16")

_module_cache = {}


def _build_module(cfg=CONFIG):
    logit_name, mlp_name = cfg
    from contextlib import ExitStack

    import concourse.bacc as bacc
    import concourse.bass as bass
    import concourse.mybir as mybir
    import concourse.tile as tile

    f32 = mybir.dt.float32
    ldt = getattr(mybir.dt, logit_name)
    mdt = getattr(mybir.dt, mlp_name)
    AF = mybir.ActivationFunctionType
    ALU = mybir.AluOpType

    def is4(dt):
        return mybir.dt.size(dt) == 4

    nc = bacc.Bacc(trn_type="TRN2", name="mha_dp")

    # f32 x always present (residual source; logit source when ldt is 4-byte)
    x_d = nc.dram_tensor("x", (C, T), f32, kind="ExternalInput")
    # narrow copies of x per 2-byte matmul family in use
    xnarrow = {}
    for dt_ in {d for d in (ldt, mdt) if mybir.dt.size(d) == 2}:
        xnarrow[dt_] = nc.dram_tensor(f"x_{dt_.name}", (C, T), dt_, kind="ExternalInput")

    wqk_d = nc.dram_tensor("wqk", (H, P, CT, 256), f32 if is4(ldt) else ldt, kind="ExternalInput")
    w1wv_d = nc.dram_tensor("w1wv", (P, CT, H * P), f32 if is4(mdt) else mdt, kind="ExternalInput")
    w1t_d = nc.dram_tensor("w1t", (P, CT, P), f32 if is4(mdt) else mdt, kind="ExternalInput")
    w2t_d = nc.dram_tensor("w2t", (P, P), f32 if is4(mdt) else mdt, kind="ExternalInput")
    ones_d = nc.dram_tensor("ones", (P, P), f32 if is4(mdt) else mdt, kind="ExternalInput")
    bqk_d = nc.dram_tensor("bqk", (H, 2, P), f32, kind="ExternalInput")
    b1e_d = nc.dram_tensor("b1e", (H, P), f32, kind="ExternalInput")
    b2_d = nc.dram_tensor("b2", (P,), f32, kind="ExternalInput")
    gam_d = nc.dram_tensor("gam", (H,), f32, kind="ExternalInput")
    out_d = nc.dram_tensor("out", (C, T), f32, kind="ExternalOutput")

    def load(sb_ap, dram_ap, dt):
        # 4-byte matmul dtypes view the f32 bytes in place; 2-byte comes
        # from host-cast arrays whose DRAM dtype already matches.
        if is4(dt):
            nc.sync.dma_start(out=sb_ap, in_=dram_ap.bitcast(dt))
        else:
            nc.sync.dma_start(out=sb_ap, in_=dram_ap)

    def mm(ps, lhsT, rhs, start, stop):
        nc.tensor.matmul(ps, lhsT, rhs, start=start, stop=stop)

    with tile.TileContext(nc) as tc, ExitStack() as ctx:
        consts = ctx.enter_context(tc.tile_pool(name="consts", bufs=1))
        psA = ctx.enter_context(tc.tile_pool(name="psA", bufs=3, space="PSUM"))
        psB = ctx.enter_context(tc.tile_pool(name="psB", bufs=1, space="PSUM"))

        wqkp = ctx.enter_context(tc.tile_pool(name="wqkp", bufs=2))
        qkp = ctx.enter_context(tc.tile_pool(name="qkp", bufs=2))
        expp = ctx.enter_context(tc.tile_pool(name="expp", bufs=3))
        hbuf = ctx.enter_context(tc.tile_pool(name="hbuf", bufs=2))
        outp = ctx.enter_context(tc.tile_pool(name="outp", bufs=2))

        # ---------------- constants + phase A (per-batch, head-independent)
        # DMA priority order: tensors gating the first matmuls come first.
        w1t_sb = consts.tile([P, CT, P], mdt, name="w1t_sb")
        load(w1t_sb, w1t_d[:], mdt)
        if mlp_name == logit_name:
            xm_sb = consts.tile([P, CT, T], mdt, name="xm_sb")
            for ci in range(CT):
                src = x_d if is4(mdt) else xnarrow[mdt]
                load(xm_sb[:, ci, :], src[ci * P : (ci + 1) * P, :], mdt)
            xl_sb = xm_sb
        else:
            xm_sb = consts.tile([P, CT, T], mdt, name="xm_sb")
            for ci in range(CT):
                src = x_d if is4(mdt) else xnarrow[mdt]
                load(xm_sb[:, ci, :], src[ci * P : (ci + 1) * P, :], mdt)
        w1wv_sb = consts.tile([P, CT, H * P], mdt, name="w1wv_sb")
        for ci in range(CT):
            load(w1wv_sb[:, ci, :], w1wv_d[:, ci, :], mdt)
        if mlp_name != logit_name:
            xl_sb = consts.tile([P, CT, T], ldt, name="xl_sb")
            for ci in range(CT):
                src = x_d if is4(ldt) else xnarrow[ldt]
                load(xl_sb[:, ci, :], src[ci * P : (ci + 1) * P, :], ldt)
        w2t_sb = consts.tile([P, P], mdt, name="w2t_sb")
        load(w2t_sb, w2t_d[:], mdt)
        ones_sb = consts.tile([P, P], mdt, name="ones_sb")
        load(ones_sb, ones_d[:], mdt)
        b2_sb = consts.tile([P, 1], f32, name="b2_sb")
        nc.sync.dma_start(out=b2_sb, in_=b2_d[:])

        xw1_sb = consts.tile([P, T], f32, name="xw1_sb")
        vw1t_sb = consts.tile([P, ST, H * P], mdt, name="vw1t_sb")

        # ---------------- per-head pipeline, software-pipelined emission.
        # PE executes its queue in order, so a chunk's consumers must be
        # emitted AFTER the next chunk's independent matmuls or the PE idles
        # waiting on the ACT/DVE chain. Stages per chunk c=(h,t2):
        #   S1(c): eT matmuls + exp          (emitted at c)
        #   S2(c): oW1 + Z matmuls            (emitted at c+1)
        #   S3(c): DVE normalize chain -> fc1 (emitted at c+1)
        #   S4(c): FC2 matmul                 (emitted at c+2)
        #   S5(c): out relu + residual add    (emitted at c+2)
        head_state = {}
        chunk_state = {}

        def emit_head_setup(h):
            wqk_sb = wqkp.tile([P, CT, 256], ldt, name="wqk_sb", tag="wqk")
            load(wqk_sb, wqk_d[h], ldt)
            bq_sb = wqkp.tile([P, 1], f32, name="bq_sb", tag="bq")
            nc.sync.dma_start(out=bq_sb, in_=bqk_d[h, 0, :])
            bk_sb = wqkp.tile([P, 1], f32, name="bk_sb", tag="bk")
            nc.sync.dma_start(out=bk_sb, in_=bqk_d[h, 1, :])
            b1e_sb = wqkp.tile([P, 1], f32, name="b1e_sb", tag="b1e")
            nc.sync.dma_start(out=b1e_sb, in_=b1e_d[h, :])
            gam_sb = wqkp.tile([P, 1], f32, name="gam_sb", tag="gam")
            gam_ap = gam_d[h : h + 1]
            nc.gpsimd.dma_start(
                out=gam_sb,
                in_=bass.AP(tensor=gam_ap.tensor, offset=gam_ap.offset, ap=[[0, P], [1, 1]]),
            )
            # residual rows h, h+8, ..., h+8*127 of x
            xres_sb = wqkp.tile([P, T], f32, name="xres_sb", tag="xres")
            x_all = x_d[:]
            nc.sync.dma_start(
                out=xres_sb,
                in_=bass.AP(tensor=x_all.tensor, offset=h * T, ap=[[H * T, P], [1, T]]),
            )
            q_sb = qkp.tile([P, T], ldt, name="q_sb", tag="q")
            k_sb = qkp.tile([P, T], ldt, name="k_sb", tag="k")
            out_sb = outp.tile([P, T], f32, name="out_sb", tag="ob")
            # K then Q, both chunks each; bias-adds hide under later MM groups
            for t2 in range(NT):
                tsl = slice(t2 * TCW, (t2 + 1) * TCW)
                ps_k = psB.tile([P, TCW], f32, name="ps_k", tag="qk", bufs=2)
                for ci in range(CT):
                    mm(ps_k, wqk_sb[:, ci, P : 2 * P], xl_sb[:, ci, tsl], ci == 0, ci == CT - 1)
                nc.vector.tensor_scalar_add(out=k_sb[:, tsl], in0=ps_k, scalar1=bk_sb)
            for t2 in range(NT):
                tsl = slice(t2 * TCW, (t2 + 1) * TCW)
                ps_q = psB.tile([P, TCW], f32, name="ps_q", tag="qk", bufs=2)
                for ci in range(CT):
                    mm(ps_q, wqk_sb[:, ci, 0:P], xl_sb[:, ci, tsl], ci == 0, ci == CT - 1)
                nc.vector.tensor_scalar_add(out=q_sb[:, tsl], in0=ps_q, scalar1=bq_sb)
            head_state[h] = dict(
                q=q_sb, k=k_sb, gam=gam_sb, b1e=b1e_sb, xres=xres_sb, out=out_sb
            )

        def emit_s1(c):
            h, t2 = c
            hs = head_state[h]
            tsl = slice(t2 * TCW, (t2 + 1) * TCW)
            et_sb = expp.tile([P, ST, TCW], mdt, name="et_sb", tag="exp")
            for si in range(ST):
                ps_e = psA.tile([P, TCW], f32, name="ps_e", tag="acc")
                mm(ps_e, hs["k"][:, si * P : (si + 1) * P], hs["q"][:, tsl], True, True)
                nc.scalar.activation(out=et_sb[:, si, :], in_=ps_e, func=AF.Exp)
            chunk_state[c] = dict(et=et_sb)

        def emit_s2_s3(c):
            h, t2 = c
            hs = head_state[h]
            cs = chunk_state[c]
            tsl = slice(t2 * TCW, (t2 + 1) * TCW)
            et_sb = cs["et"]
            ps_o = psB.tile([P, TCW], f32, name="ps_o", tag="oo")
            for si in range(ST):
                mm(
                    ps_o,
                    vw1t_sb[:, si, h * P : (h + 1) * P],
                    et_sb[:, si, :],
                    si == 0,
                    si == ST - 1,
                )
            # Z: tree-sum the 8 s-tiles on DVE (free-dim adds), then a single
            # ones-matmul for the partition reduction + broadcast.
            r1 = hbuf.tile([P, 4, TCW], f32, name="r1", tag="r1")
            nc.vector.tensor_add(r1, et_sb[:, 0:4, :], et_sb[:, 4:8, :])
            r2 = hbuf.tile([P, 2, TCW], f32, name="r2", tag="r2")
            nc.vector.tensor_add(r2, r1[:, 0:2, :], r1[:, 2:4, :])
            etsum = hbuf.tile([P, TCW], mdt, name="etsum", tag="etsum")
            nc.vector.tensor_add(etsum, r2[:, 0, :], r2[:, 1, :])
            ps_z = psB.tile([P, TCW], f32, name="ps_z", tag="zz")
            mm(ps_z, ones_sb, etsum, True, True)
            # fc1 = relu(gamma * oW1/Z + xW1 + b1eff), all on DVE
            izg = hbuf.tile([P, TCW], f32, name="izg", tag="izg")
            nc.vector.reciprocal_approx_fast(out=izg, in_=ps_z)
            t1 = hbuf.tile([P, TCW], f32, name="t1", tag="t1")
            nc.vector.scalar_tensor_tensor(
                out=t1, in0=ps_o, scalar=hs["gam"], in1=izg, op0=ALU.mult, op1=ALU.mult
            )
            t2t = hbuf.tile([P, TCW], f32, name="t2t", tag="t2t")
            nc.vector.scalar_tensor_tensor(
                out=t2t, in0=t1, scalar=hs["b1e"], in1=xw1_sb[:, tsl], op0=ALU.add, op1=ALU.add
            )
            fc1 = hbuf.tile([P, TCW], mdt, name="fc1", tag="fc1")
            nc.vector.tensor_scalar_max(out=fc1, in0=t2t, scalar1=0.0)
            cs["fc1"] = fc1

        def emit_s4_s5(c):
            h, t2 = c
            hs = head_state[h]
            cs = chunk_state[c]
            tsl = slice(t2 * TCW, (t2 + 1) * TCW)
            ps_f = psB.tile([P, TCW], f32, name="ps_f", tag="ff")
            mm(ps_f, w2t_sb[:], cs["fc1"], True, True)
            ot = hbuf.tile([P, TCW], f32, name="ot", tag="ot")
            nc.scalar.activation(out=ot, in_=ps_f, func=AF.Relu, bias=b2_sb)
            nc.vector.tensor_add(hs["out"][:, tsl], ot, hs["xres"][:, tsl])
            out_all = out_d[:]
            nc.sync.dma_start(
                out=bass.AP(
                    tensor=out_all.tensor,
                    offset=h * T + t2 * TCW,
                    ap=[[H * T, P], [1, TCW]],
                ),
                in_=hs["out"][:, tsl],
            )

        # ---- phase A compute: xW1 first (smallest DMA deps), head-0 QK
        # next (overlaps w1wv DMA), then vW1T.
        for t2 in range(NT):
            tsl = slice(t2 * TCW, (t2 + 1) * TCW)
            ps_x = psA.tile([P, TCW], f32, name="ps_x", tag="acc")
            for ci in range(CT):
                mm(ps_x, w1t_sb[:, ci, :], xm_sb[:, ci, tsl], ci == 0, ci == CT - 1)
            nc.vector.tensor_copy(out=xw1_sb[:, tsl], in_=ps_x)

        # vW1T = x.T @ W1WvT for all heads  (s on partitions, h*128+j free)
        for si in range(ST):
            for jh in range(2):
                jsl = slice(jh * 512, (jh + 1) * 512)
                ps_v = psA.tile([P, TCW], f32, name="ps_v", tag="acc")
                for ci in range(CT):
                    mm(
                        ps_v,
                        xm_sb[:, ci, si * P : (si + 1) * P],
                        w1wv_sb[:, ci, jsl],
                        ci == 0,
                        ci == CT - 1,
                    )
                nc.vector.tensor_copy(out=vw1t_sb[:, si, jsl], in_=ps_v)

        chunks = [(h, t2) for h in range(H) for t2 in range(NT)]
        for i, c in enumerate(chunks):
            if c[1] == 0:
                emit_head_setup(c[0])
            emit_s1(c)
            if i >= 1:
                emit_s2_s3(chunks[i - 1])
            if i >= 2:
                emit_s4_s5(chunks[i - 2])
        emit_s2_s3(chunks[-1])
        emit_s4_s5(chunks[-2])
        emit_s4_s5(chunks[-1])

    nc.compile()
    return nc


def _prepare_inputs(inputs, cfg=CONFIG):
    import ml_dtypes

    logit_name, mlp_name = cfg
    l4 = logit_name in ("float32", "float32r")
    m4 = mlp_name in ("float32", "float32r")
    np_dt = {"bfloat16": ml_dtypes.bfloat16, "float16": np.float16}

    x = np.ascontiguousarray(np.asarray(inputs["x"], dtype=np.float32))
    Wq = np.asarray(inputs["Wq"], dtype=np.float32)
    bq = np.asarray(inputs["bq"], dtype=np.float32)
    Wk = np.asarray(inputs["Wk"], dtype=np.float32)
    bk = np.asarray(inputs["bk"], dtype=np.float32)
    Wv = np.asarray(inputs["Wv"], dtype=np.float32)
    bv = np.asarray(inputs["bv"], dtype=np.float32)
    gamma = np.asarray(inputs["gamma"], dtype=np.float32)
    W1 = np.asarray(inputs["W1"], dtype=np.float32)
    b1 = np.asarray(inputs["b1"], dtype=np.float32)
    W2 = np.asarray(inputs["W2"], dtype=np.float32)
    b2 = np.asarray(inputs["b2"], dtype=np.float32)

    # wqk[h, cp, ci, 0:128] = Wq[h].T[ci*128+cp, :]; 128:256 for Wk
    wqk = np.empty((H, P, CT, 256), dtype=np.float32)
    for h in range(H):
        wqk[h, :, :, 0:P] = Wq[h].T.reshape(CT, P, P).transpose(1, 0, 2)
        wqk[h, :, :, P : 2 * P] = Wk[h].T.reshape(CT, P, P).transpose(1, 0, 2)

    # w1wv[cp, ci, h*128+j] = (W1 @ Wv[h]).T[ci*128+cp, j]
    w1wvT = np.concatenate([(W1 @ Wv[h]).T for h in range(H)], axis=1)  # (C, H*128)
    w1wv = np.ascontiguousarray(w1wvT.reshape(CT, P, H * P).transpose(1, 0, 2))

    w1t = np.ascontiguousarray(W1.T.reshape(CT, P, P).transpose(1, 0, 2))
    w2t = np.ascontiguousarray(W2.T)

    bqk = np.stack([bq, bk], axis=1)  # (H, 2, P)
    b1v = bv @ W1.T  # (H, P): b1v[h] = W1 @ bv[h]
    b1e = b1[None, :] + gamma[:, None] * b1v  # (H, P)

    def fam(arr, four_byte, name):
        arr = np.ascontiguousarray(arr, dtype=np.float32)
        return arr if four_byte else np.ascontiguousarray(arr.astype(np_dt[name]))

    shared = {
        "wqk": fam(wqk, l4, logit_name),
        "w1wv": fam(w1wv, m4, mlp_name),
        "w1t": fam(w1t, m4, mlp_name),
        "w2t": fam(w2t, m4, mlp_name),
        "ones": fam(np.ones((P, P), dtype=np.float32), m4, mlp_name),
        "bqk": np.ascontiguousarray(bqk),
        "b1e": np.ascontiguousarray(b1e),
        "b2": np.ascontiguousarray(b2),
        "gam": np.ascontiguousarray(gamma),
    }
    narrow_names = {n for n, four in ((logit_name, l4), (mlp_name, m4)) if not four}
    in_maps = []
    for b in range(B):
        m = dict(shared)
        m["x"] = np.ascontiguousarray(x[b])
        for n in narrow_names:
            m[f"x_{n}"] = np.ascontiguousarray(x[b].astype(np_dt[n]))
        in_maps.append(m)
    return in_maps


def kernel(**inputs):
    from concourse.bass_utils import run_bass_kernel_spmd

    if CONFIG not in _module_cache:
        _module_cache[CONFIG] = _build_module(CONFIG)
    nc = _module_cache[CONFIG]

    in_maps = _prepare_inputs(inputs, CONFIG)
    res = run_bass_kernel_spmd(nc, in_maps, core_ids=list(range(B)))
    out = np.stack([res.results[b]["out"] for b in range(B)], axis=0)
    return out.astype(np.float32)


# revision 17
# speedup vs baseline: 1.1821x; 1.1017x over previous
"""Trainium2 Bass kernel for nn_MultiHeadAttention_9491877724818.

Math (per batch b, head h), reformulated from the reference:
    q = Wq_h @ x_b + bq          (128, T)
    k = Wk_h @ x_b + bk          (128, T)
    eT[s,t] = (k.T @ q)[s,t]     == energy[t,s]; softmax over s (partition dim)
    expET = exp(eT)              (no max subtraction: |logit| <= ~70, fp32-safe)
    Z[t] = sum_s expET[s,t]      (PE ones-matmul -> broadcast across partitions)
Key algebraic folding: heads only enter the output through W1 (DFC1=128 rows),
so the huge Wv (C x C) conv and o = v @ attn (each 2.1 GF/bh) collapse into
128-channel products:
    vW1T[s,j]  = (x_b.T @ (W1 @ Wv_h).T)[s,j]          (T, 128)
    oW1raw[j,t]= sum_s vW1T[s,j] expET[s,t]            (128, T)
    fc1[j,t]   = relu(gamma_h * oW1raw[j,t]/Z[t] + xW1[b][j,t] + b1eff_h[j])
        where xW1 = W1 @ x_b, b1eff = b1 + gamma_h * (W1 @ bv_h)
        (softmax rows sum to 1 => v-bias passes through as a constant)
    out2[d,t]  = relu(W2 @ fc1 + b2)
    final[b, 8*d + h, t] = out2[d,t] + x[b, 8*d + h, t]

Sharding: data parallel - core i computes batch b=i entirely (all 8 heads).

Dtypes: two matmul families. The logit path (QK convs + k.T@q) needs accuracy
because exp() amplifies absolute logit error; the post-softmax path is plain
linear algebra where bf16 (~0.2% rel) is fine. float32r = fp32 storage with
reduced-precision PE multiply (~2 cyc/row measured); bf16 = 1 cyc/row.
"""

import numpy as np

B, C, T, H, P = 8, 1024, 1024, 8, 128
CT = C // P      # 8 contraction k-tiles over channels
ST = T // P      # 8 s-tiles (softmax/partition dim)
NT = 2           # t-chunks per row
TCW = T // NT    # 512 = matmul moving free dim

# (logit_dt, mlp_dt)
CONFIG = ("float16", "bfloat16")

_module_cache = {}


def _build_module(cfg=CONFIG):
    logit_name, mlp_name = cfg
    from contextlib import ExitStack

    import concourse.bacc as bacc
    import concourse.bass as bass
    import concourse.mybir as mybir
    import concourse.tile as tile

    f32 = mybir.dt.float32
    ldt = getattr(mybir.dt, logit_name)
    mdt = getattr(mybir.dt, mlp_name)
    AF = mybir.ActivationFunctionType
    ALU = mybir.AluOpType

    def is4(dt):
        return mybir.dt.size(dt) == 4

    nc = bacc.Bacc(trn_type="TRN2", name="mha_dp")

    # f32 x always present (residual source; logit source when ldt is 4-byte)
    x_d = nc.dram_tensor("x", (C, T), f32, kind="ExternalInput")
    # narrow copies of x per 2-byte matmul family in use
    xnarrow = {}
    for dt_ in {d for d in (ldt, mdt) if mybir.dt.size(d) == 2}:
        xnarrow[dt_] = nc.dram_tensor(f"x_{dt_.name}", (C, T), dt_, kind="ExternalInput")

    wqk_d = nc.dram_tensor("wqk", (H, P, CT, 256), f32 if is4(ldt) else ldt, kind="ExternalInput")
    w1wv_d = nc.dram_tensor("w1wv", (P, CT, H * P), f32 if is4(mdt) else mdt, kind="ExternalInput")
    w1t_d = nc.dram_tensor("w1t", (P, CT, P), f32 if is4(mdt) else mdt, kind="ExternalInput")
    w2t_d = nc.dram_tensor("w2t", (P, P), f32 if is4(mdt) else mdt, kind="ExternalInput")
    ones_d = nc.dram_tensor("ones", (P, P), f32 if is4(mdt) else mdt, kind="ExternalInput")
    bqk_d = nc.dram_tensor("bqk", (H, 2, P), f32, kind="ExternalInput")
    b1e_d = nc.dram_tensor("b1e", (H, P), f32, kind="ExternalInput")
    b2_d = nc.dram_tensor("b2", (P,), f32, kind="ExternalInput")
    gam_d = nc.dram_tensor("gam", (H,), f32, kind="ExternalInput")
    out_d = nc.dram_tensor("out", (C, T), f32, kind="ExternalOutput")

    def load(sb_ap, dram_ap, dt):
        # 4-byte matmul dtypes view the f32 bytes in place; 2-byte comes
        # from host-cast arrays whose DRAM dtype already matches.
        if is4(dt):
            nc.sync.dma_start(out=sb_ap, in_=dram_ap.bitcast(dt))
        else:
            nc.sync.dma_start(out=sb_ap, in_=dram_ap)

    def mm(ps, lhsT, rhs, start, stop):
        nc.tensor.matmul(ps, lhsT, rhs, start=start, stop=stop)

    with tile.TileContext(nc) as tc, ExitStack() as ctx:
        consts = ctx.enter_context(tc.tile_pool(name="consts", bufs=1))
        psA = ctx.enter_context(tc.tile_pool(name="psA", bufs=3, space="PSUM"))
        psB = ctx.enter_context(tc.tile_pool(name="psB", bufs=1, space="PSUM"))

        wqkp = ctx.enter_context(tc.tile_pool(name="wqkp", bufs=2))
        qkp = ctx.enter_context(tc.tile_pool(name="qkp", bufs=2))
        expp = ctx.enter_context(tc.tile_pool(name="expp", bufs=3))
        hbuf = ctx.enter_context(tc.tile_pool(name="hbuf", bufs=2))
        outp = ctx.enter_context(tc.tile_pool(name="outp", bufs=2))

        # ---------------- constants + phase A (per-batch, head-independent)
        # DMA priority order: tensors gating the first matmuls come first.
        w1t_sb = consts.tile([P, CT, P], mdt, name="w1t_sb")
        load(w1t_sb, w1t_d[:], mdt)
        if mlp_name == logit_name:
            xm_sb = consts.tile([P, CT, T], mdt, name="xm_sb")
            for ci in range(CT):
                src = x_d if is4(mdt) else xnarrow[mdt]
                load(xm_sb[:, ci, :], src[ci * P : (ci + 1) * P, :], mdt)
            xl_sb = xm_sb
        else:
            xm_sb = consts.tile([P, CT, T], mdt, name="xm_sb")
            for ci in range(CT):
                src = x_d if is4(mdt) else xnarrow[mdt]
                load(xm_sb[:, ci, :], src[ci * P : (ci + 1) * P, :], mdt)
        w1wv_sb = consts.tile([P, CT, H * P], mdt, name="w1wv_sb")
        for ci in range(CT):
            load(w1wv_sb[:, ci, :], w1wv_d[:, ci, :], mdt)
        if mlp_name != logit_name:
            xl_sb = consts.tile([P, CT, T], ldt, name="xl_sb")
            for ci in range(CT):
                src = x_d if is4(ldt) else xnarrow[ldt]
                load(xl_sb[:, ci, :], src[ci * P : (ci + 1) * P, :], ldt)
        w2t_sb = consts.tile([P, P], mdt, name="w2t_sb")
        load(w2t_sb, w2t_d[:], mdt)
        ones_sb = consts.tile([P, P], mdt, name="ones_sb")
        load(ones_sb, ones_d[:], mdt)
        b2_sb = consts.tile([P, 1], f32, name="b2_sb")
        nc.sync.dma_start(out=b2_sb, in_=b2_d[:])

        xw1_sb = consts.tile([P, T], f32, name="xw1_sb")
        vw1t_sb = consts.tile([P, ST, H * P], mdt, name="vw1t_sb")

        # ---------------- per-head pipeline, software-pipelined emission.
        # PE executes its queue in order, so a chunk's consumers must be
        # emitted AFTER the next chunk's independent matmuls or the PE idles
        # waiting on the ACT/DVE chain. Stages per chunk c=(h,t2):
        #   S1(c): eT matmuls + exp          (emitted at c)
        #   S2(c): oW1 + Z matmuls            (emitted at c+1)
        #   S3(c): DVE normalize chain -> fc1 (emitted at c+1)
        #   S4(c): FC2 matmul                 (emitted at c+2)
        #   S5(c): out relu + residual add    (emitted at c+2)
        head_state = {}
        chunk_state = {}

        def emit_head_setup(h):
            wqk_sb = wqkp.tile([P, CT, 256], ldt, name="wqk_sb", tag="wqk")
            load(wqk_sb, wqk_d[h], ldt)
            bq_sb = wqkp.tile([P, 1], f32, name="bq_sb", tag="bq")
            nc.sync.dma_start(out=bq_sb, in_=bqk_d[h, 0, :])
            bk_sb = wqkp.tile([P, 1], f32, name="bk_sb", tag="bk")
            nc.sync.dma_start(out=bk_sb, in_=bqk_d[h, 1, :])
            b1e_sb = wqkp.tile([P, 1], f32, name="b1e_sb", tag="b1e")
            nc.sync.dma_start(out=b1e_sb, in_=b1e_d[h, :])
            gam_sb = wqkp.tile([P, 1], f32, name="gam_sb", tag="gam")
            gam_ap = gam_d[h : h + 1]
            nc.gpsimd.dma_start(
                out=gam_sb,
                in_=bass.AP(tensor=gam_ap.tensor, offset=gam_ap.offset, ap=[[0, P], [1, 1]]),
            )
            # residual rows h, h+8, ..., h+8*127 of x
            xres_sb = wqkp.tile([P, T], f32, name="xres_sb", tag="xres")
            x_all = x_d[:]
            nc.sync.dma_start(
                out=xres_sb,
                in_=bass.AP(tensor=x_all.tensor, offset=h * T, ap=[[H * T, P], [1, T]]),
            )
            q_sb = qkp.tile([P, T], ldt, name="q_sb", tag="q")
            k_sb = qkp.tile([P, T], ldt, name="k_sb", tag="k")
            out_sb = outp.tile([P, T], f32, name="out_sb", tag="ob")
            # K then Q, both chunks each; bias-adds hide under later MM groups
            for t2 in range(NT):
                tsl = slice(t2 * TCW, (t2 + 1) * TCW)
                ps_k = psB.tile([P, TCW], f32, name="ps_k", tag="qk", bufs=2)
                for ci in range(CT):
                    mm(ps_k, wqk_sb[:, ci, P : 2 * P], xl_sb[:, ci, tsl], ci == 0, ci == CT - 1)
                nc.vector.tensor_scalar_add(out=k_sb[:, tsl], in0=ps_k, scalar1=bk_sb)
            for t2 in range(NT):
                tsl = slice(t2 * TCW, (t2 + 1) * TCW)
                ps_q = psB.tile([P, TCW], f32, name="ps_q", tag="qk", bufs=2)
                for ci in range(CT):
                    mm(ps_q, wqk_sb[:, ci, 0:P], xl_sb[:, ci, tsl], ci == 0, ci == CT - 1)
                nc.vector.tensor_scalar_add(out=q_sb[:, tsl], in0=ps_q, scalar1=bq_sb)
            head_state[h] = dict(
                q=q_sb, k=k_sb, gam=gam_sb, b1e=b1e_sb, xres=xres_sb, out=out_sb
            )

        def emit_s1(c):
            h, t2 = c
            hs = head_state[h]
            tsl = slice(t2 * TCW, (t2 + 1) * TCW)
            et_sb = expp.tile([P, ST, TCW], mdt, name="et_sb", tag="exp")
            for si in range(ST):
                ps_e = psA.tile([P, TCW], f32, name="ps_e", tag="acc")
                mm(ps_e, hs["k"][:, si * P : (si + 1) * P], hs["q"][:, tsl], True, True)
                nc.scalar.activation(out=et_sb[:, si, :], in_=ps_e, func=AF.Exp)
            chunk_state[c] = dict(et=et_sb)

        def emit_s2_s3(c):
            h, t2 = c
            hs = head_state[h]
            cs = chunk_state[c]
            tsl = slice(t2 * TCW, (t2 + 1) * TCW)
            et_sb = cs["et"]
            ps_o = psB.tile([P, TCW], f32, name="ps_o", tag="oo")
            for si in range(ST):
                mm(
                    ps_o,
                    vw1t_sb[:, si, h * P : (h + 1) * P],
                    et_sb[:, si, :],
                    si == 0,
                    si == ST - 1,
                )
            # Z: tree-sum the 8 s-tiles on DVE (free-dim adds), then a single
            # ones-matmul for the partition reduction + broadcast.
            r1 = hbuf.tile([P, 4, TCW], mdt, name="r1", tag="r1")
            nc.vector.tensor_add(r1, et_sb[:, 0:4, :], et_sb[:, 4:8, :])
            r2 = hbuf.tile([P, 2, TCW], mdt, name="r2", tag="r2")
            nc.vector.tensor_add(r2, r1[:, 0:2, :], r1[:, 2:4, :])
            etsum = hbuf.tile([P, TCW], mdt, name="etsum", tag="etsum")
            nc.vector.tensor_add(etsum, r2[:, 0, :], r2[:, 1, :])
            ps_z = psB.tile([P, TCW], f32, name="ps_z", tag="zz")
            mm(ps_z, ones_sb, etsum, True, True)
            # fc1 = relu(gamma * oW1/Z + xW1 + b1eff), all on DVE
            izg = hbuf.tile([P, TCW], f32, name="izg", tag="izg")
            nc.vector.reciprocal_approx_fast(out=izg, in_=ps_z)
            t1 = hbuf.tile([P, TCW], f32, name="t1", tag="t1")
            nc.vector.scalar_tensor_tensor(
                out=t1, in0=ps_o, scalar=hs["gam"], in1=izg, op0=ALU.mult, op1=ALU.mult
            )
            t2t = hbuf.tile([P, TCW], f32, name="t2t", tag="t2t")
            nc.vector.scalar_tensor_tensor(
                out=t2t, in0=t1, scalar=hs["b1e"], in1=xw1_sb[:, tsl], op0=ALU.add, op1=ALU.add
            )
            fc1 = hbuf.tile([P, TCW], mdt, name="fc1", tag="fc1")
            nc.vector.tensor_scalar_max(out=fc1, in0=t2t, scalar1=0.0)
            cs["fc1"] = fc1

        def emit_s4_s5(c):
            h, t2 = c
            hs = head_state[h]
            cs = chunk_state[c]
            tsl = slice(t2 * TCW, (t2 + 1) * TCW)
            ps_f = psB.tile([P, TCW], f32, name="ps_f", tag="ff")
            mm(ps_f, w2t_sb[:], cs["fc1"], True, True)
            ot = hbuf.tile([P, TCW], f32, name="ot", tag="ot")
            nc.scalar.activation(out=ot, in_=ps_f, func=AF.Relu, bias=b2_sb)
            nc.vector.tensor_add(hs["out"][:, tsl], ot, hs["xres"][:, tsl])
            out_all = out_d[:]
            nc.sync.dma_start(
                out=bass.AP(
                    tensor=out_all.tensor,
                    offset=h * T + t2 * TCW,
                    ap=[[H * T, P], [1, TCW]],
                ),
                in_=hs["out"][:, tsl],
            )

        # ---- phase A compute: xW1 first (smallest DMA deps), head-0 QK
        # next (overlaps w1wv DMA), then vW1T.
        for t2 in range(NT):
            tsl = slice(t2 * TCW, (t2 + 1) * TCW)
            ps_x = psA.tile([P, TCW], f32, name="ps_x", tag="acc")
            for ci in range(CT):
                mm(ps_x, w1t_sb[:, ci, :], xm_sb[:, ci, tsl], ci == 0, ci == CT - 1)
            nc.vector.tensor_copy(out=xw1_sb[:, tsl], in_=ps_x)

        # vW1T = x.T @ W1WvT for all heads  (s on partitions, h*128+j free)
        for si in range(ST):
            for jh in range(2):
                jsl = slice(jh * 512, (jh + 1) * 512)
                ps_v = psA.tile([P, TCW], f32, name="ps_v", tag="acc")
                for ci in range(CT):
                    mm(
                        ps_v,
                        xm_sb[:, ci, si * P : (si + 1) * P],
                        w1wv_sb[:, ci, jsl],
                        ci == 0,
                        ci == CT - 1,
                    )
                nc.vector.tensor_copy(out=vw1t_sb[:, si, jsl], in_=ps_v)

        chunks = [(h, t2) for h in range(H) for t2 in range(NT)]
        for i, c in enumerate(chunks):
            if c[1] == 0:
                emit_head_setup(c[0])
            emit_s1(c)
            if i >= 1:
                emit_s2_s3(chunks[i - 1])
            if i >= 2:
                emit_s4_s5(chunks[i - 2])
        emit_s2_s3(chunks[-1])
        emit_s4_s5(chunks[-2])
        emit_s4_s5(chunks[-1])

    nc.compile()
    return nc


def _prepare_inputs(inputs, cfg=CONFIG):
    import ml_dtypes

    logit_name, mlp_name = cfg
    l4 = logit_name in ("float32", "float32r")
    m4 = mlp_name in ("float32", "float32r")
    np_dt = {"bfloat16": ml_dtypes.bfloat16, "float16": np.float16}

    x = np.ascontiguousarray(np.asarray(inputs["x"], dtype=np.float32))
    Wq = np.asarray(inputs["Wq"], dtype=np.float32)
    bq = np.asarray(inputs["bq"], dtype=np.float32)
    Wk = np.asarray(inputs["Wk"], dtype=np.float32)
    bk = np.asarray(inputs["bk"], dtype=np.float32)
    Wv = np.asarray(inputs["Wv"], dtype=np.float32)
    bv = np.asarray(inputs["bv"], dtype=np.float32)
    gamma = np.asarray(inputs["gamma"], dtype=np.float32)
    W1 = np.asarray(inputs["W1"], dtype=np.float32)
    b1 = np.asarray(inputs["b1"], dtype=np.float32)
    W2 = np.asarray(inputs["W2"], dtype=np.float32)
    b2 = np.asarray(inputs["b2"], dtype=np.float32)

    # wqk[h, cp, ci, 0:128] = Wq[h].T[ci*128+cp, :]; 128:256 for Wk
    wqk = np.empty((H, P, CT, 256), dtype=np.float32)
    for h in range(H):
        wqk[h, :, :, 0:P] = Wq[h].T.reshape(CT, P, P).transpose(1, 0, 2)
        wqk[h, :, :, P : 2 * P] = Wk[h].T.reshape(CT, P, P).transpose(1, 0, 2)

    # w1wv[cp, ci, h*128+j] = (W1 @ Wv[h]).T[ci*128+cp, j]
    w1wvT = np.concatenate([(W1 @ Wv[h]).T for h in range(H)], axis=1)  # (C, H*128)
    w1wv = np.ascontiguousarray(w1wvT.reshape(CT, P, H * P).transpose(1, 0, 2))

    w1t = np.ascontiguousarray(W1.T.reshape(CT, P, P).transpose(1, 0, 2))
    w2t = np.ascontiguousarray(W2.T)

    bqk = np.stack([bq, bk], axis=1)  # (H, 2, P)
    b1v = bv @ W1.T  # (H, P): b1v[h] = W1 @ bv[h]
    b1e = b1[None, :] + gamma[:, None] * b1v  # (H, P)

    def fam(arr, four_byte, name):
        arr = np.ascontiguousarray(arr, dtype=np.float32)
        return arr if four_byte else np.ascontiguousarray(arr.astype(np_dt[name]))

    shared = {
        "wqk": fam(wqk, l4, logit_name),
        "w1wv": fam(w1wv, m4, mlp_name),
        "w1t": fam(w1t, m4, mlp_name),
        "w2t": fam(w2t, m4, mlp_name),
        "ones": fam(np.ones((P, P), dtype=np.float32), m4, mlp_name),
        "bqk": np.ascontiguousarray(bqk),
        "b1e": np.ascontiguousarray(b1e),
        "b2": np.ascontiguousarray(b2),
        "gam": np.ascontiguousarray(gamma),
    }
    narrow_names = {n for n, four in ((logit_name, l4), (mlp_name, m4)) if not four}
    in_maps = []
    for b in range(B):
        m = dict(shared)
        m["x"] = np.ascontiguousarray(x[b])
        for n in narrow_names:
            m[f"x_{n}"] = np.ascontiguousarray(x[b].astype(np_dt[n]))
        in_maps.append(m)
    return in_maps


def kernel(**inputs):
    from concourse.bass_utils import run_bass_kernel_spmd

    if CONFIG not in _module_cache:
        _module_cache[CONFIG] = _build_module(CONFIG)
    nc = _module_cache[CONFIG]

    in_maps = _prepare_inputs(inputs, CONFIG)
    res = run_bass_kernel_spmd(nc, in_maps, core_ids=list(range(B)))
    out = np.stack([res.results[b]["out"] for b in range(B)], axis=0)
    return out.astype(np.float32)


# revision 26
# speedup vs baseline: 1.1913x; 1.0077x over previous
"""Trainium2 Bass kernel for nn_MultiHeadAttention_9491877724818.

Math (per batch b, head h), reformulated from the reference:
    q = Wq_h @ x_b + bq          (128, T)
    k = Wk_h @ x_b + bk          (128, T)
    eT[s,t] = (k.T @ q)[s,t]     == energy[t,s]; softmax over s (partition dim)
    expET = exp(eT)              (no max subtraction: |logit| <= ~70, fp32-safe)
    Z[t] = sum_s expET[s,t]      (PE ones-matmul -> broadcast across partitions)
Key algebraic folding: heads only enter the output through W1 (DFC1=128 rows),
so the huge Wv (C x C) conv and o = v @ attn (each 2.1 GF/bh) collapse into
128-channel products:
    vW1T[s,j]  = (x_b.T @ (W1 @ Wv_h).T)[s,j]          (T, 128)
    oW1raw[j,t]= sum_s vW1T[s,j] expET[s,t]            (128, T)
    fc1[j,t]   = relu(gamma_h * oW1raw[j,t]/Z[t] + xW1[b][j,t] + b1eff_h[j])
        where xW1 = W1 @ x_b, b1eff = b1 + gamma_h * (W1 @ bv_h)
        (softmax rows sum to 1 => v-bias passes through as a constant)
    out2[d,t]  = relu(W2 @ fc1 + b2)
    final[b, 8*d + h, t] = out2[d,t] + x[b, 8*d + h, t]

Sharding: data parallel - core i computes batch b=i entirely (all 8 heads).

Dtypes: two matmul families. The logit path (QK convs + k.T@q) needs accuracy
because exp() amplifies absolute logit error; the post-softmax path is plain
linear algebra where bf16 (~0.2% rel) is fine. float32r = fp32 storage with
reduced-precision PE multiply (~2 cyc/row measured); bf16 = 1 cyc/row.
"""

import numpy as np

B, C, T, H, P = 8, 1024, 1024, 8, 128
CT = C // P      # 8 contraction k-tiles over channels
ST = T // P      # 8 s-tiles (softmax/partition dim)
NT = 2           # t-chunks per row
TCW = T // NT    # 512 = matmul moving free dim

# (logit_dt, mlp_dt)
CONFIG = ("float16", "bfloat16")

_module_cache = {}


def _build_module(cfg=CONFIG):
    logit_name, mlp_name = cfg
    from contextlib import ExitStack

    import concourse.bacc as bacc
    import concourse.bass as bass
    import concourse.mybir as mybir
    import concourse.tile as tile

    f32 = mybir.dt.float32
    ldt = getattr(mybir.dt, logit_name)
    mdt = getattr(mybir.dt, mlp_name)
    AF = mybir.ActivationFunctionType
    ALU = mybir.AluOpType

    def is4(dt):
        return mybir.dt.size(dt) == 4

    nc = bacc.Bacc(trn_type="TRN2", name="mha_dp")

    # f32 x always present (residual source; logit source when ldt is 4-byte)
    x_d = nc.dram_tensor("x", (C, T), f32, kind="ExternalInput")
    # narrow copies of x per 2-byte matmul family in use
    xnarrow = {}
    for dt_ in {d for d in (ldt, mdt) if mybir.dt.size(d) == 2}:
        xnarrow[dt_] = nc.dram_tensor(f"x_{dt_.name}", (C, T), dt_, kind="ExternalInput")

    wqk_d = nc.dram_tensor("wqk", (H, P, CT, 256), f32 if is4(ldt) else ldt, kind="ExternalInput")
    w1wv_d = nc.dram_tensor("w1wv", (P, CT, H * P), f32 if is4(mdt) else mdt, kind="ExternalInput")
    w1t_d = nc.dram_tensor("w1t", (P, CT, P), f32 if is4(mdt) else mdt, kind="ExternalInput")
    w2t_d = nc.dram_tensor("w2t", (P, P), f32 if is4(mdt) else mdt, kind="ExternalInput")
    ones_d = nc.dram_tensor("ones", (P, P), f32 if is4(mdt) else mdt, kind="ExternalInput")
    bqk_d = nc.dram_tensor("bqk", (H, 2, P), f32, kind="ExternalInput")
    b1e_d = nc.dram_tensor("b1e", (H, P), f32, kind="ExternalInput")
    b2_d = nc.dram_tensor("b2", (P,), f32, kind="ExternalInput")
    gam_d = nc.dram_tensor("gam", (H,), f32, kind="ExternalInput")
    out_d = nc.dram_tensor("out", (C, T), f32, kind="ExternalOutput")

    def load(sb_ap, dram_ap, dt, eng=None):
        # 4-byte matmul dtypes view the f32 bytes in place; 2-byte comes
        # from host-cast arrays whose DRAM dtype already matches.
        eng = eng if eng is not None else nc.sync
        if is4(dt):
            eng.dma_start(out=sb_ap, in_=dram_ap.bitcast(dt))
        else:
            eng.dma_start(out=sb_ap, in_=dram_ap)

    def mm(ps, lhsT, rhs, start, stop):
        nc.tensor.matmul(ps, lhsT, rhs, start=start, stop=stop)

    with tile.TileContext(nc) as tc, ExitStack() as ctx:
        consts = ctx.enter_context(tc.tile_pool(name="consts", bufs=1))
        psA = ctx.enter_context(tc.tile_pool(name="psA", bufs=4, space="PSUM"))
        psB = ctx.enter_context(tc.tile_pool(name="psB", bufs=1, space="PSUM"))

        wqkp = ctx.enter_context(tc.tile_pool(name="wqkp", bufs=2))
        qkp = ctx.enter_context(tc.tile_pool(name="qkp", bufs=2))
        expp = ctx.enter_context(tc.tile_pool(name="expp", bufs=3))
        hbuf = ctx.enter_context(tc.tile_pool(name="hbuf", bufs=2))
        outp = ctx.enter_context(tc.tile_pool(name="outp", bufs=2))

        # ---------------- constants + phase A (per-batch, head-independent)
        # DMA priority order: tensors gating the first matmuls come first.
        w1t_sb = consts.tile([P, CT, P], mdt, name="w1t_sb")
        load(w1t_sb, w1t_d[:], mdt)
        xm_sb = consts.tile([P, CT, T], mdt, name="xm_sb")
        for ci in range(CT):
            src = x_d if is4(mdt) else xnarrow[mdt]
            load(xm_sb[:, ci, :], src[ci * P : (ci + 1) * P, :], mdt)
        if mlp_name == logit_name:
            xl_sb = xm_sb
        w1wv_sb = consts.tile([P, CT, H * P], mdt, name="w1wv_sb")
        for ci in range(CT):
            load(w1wv_sb[:, ci, :], w1wv_d[:, ci, :], mdt)
        if mlp_name != logit_name:
            xl_sb = consts.tile([P, CT, T], ldt, name="xl_sb")
            for ci in range(CT):
                src = x_d if is4(ldt) else xnarrow[ldt]
                load(xl_sb[:, ci, :], src[ci * P : (ci + 1) * P, :], ldt)
        w2t_sb = consts.tile([P, P], mdt, name="w2t_sb")
        load(w2t_sb, w2t_d[:], mdt)
        ones_sb = consts.tile([P, P], mdt, name="ones_sb")
        load(ones_sb, ones_d[:], mdt)
        b2_sb = consts.tile([P, 1], f32, name="b2_sb")
        nc.sync.dma_start(out=b2_sb, in_=b2_d[:])

        xw1_sb = consts.tile([P, T], f32, name="xw1_sb")
        vw1t_sb = consts.tile([P, ST, H * P], mdt, name="vw1t_sb")

        # ---------------- per-head pipeline, software-pipelined emission.
        # PE executes its queue in order, so a chunk's consumers must be
        # emitted AFTER the next chunk's independent matmuls or the PE idles
        # waiting on the ACT/DVE chain. Stages per chunk c=(h,t2):
        #   S1(c): eT matmuls + exp          (emitted at c)
        #   S2(c): oW1 + Z matmuls            (emitted at c+1)
        #   S3(c): DVE normalize chain -> fc1 (emitted at c+1)
        #   S4(c): FC2 matmul                 (emitted at c+2)
        #   S5(c): out relu + residual add    (emitted at c+2)
        head_state = {}
        chunk_state = {}

        def emit_head_setup(h):
            wqk_sb = wqkp.tile([P, CT, 256], ldt, name="wqk_sb", tag="wqk")
            load(wqk_sb, wqk_d[h], ldt)
            bq_sb = wqkp.tile([P, 1], f32, name="bq_sb", tag="bq")
            nc.sync.dma_start(out=bq_sb, in_=bqk_d[h, 0, :])
            bk_sb = wqkp.tile([P, 1], f32, name="bk_sb", tag="bk")
            nc.sync.dma_start(out=bk_sb, in_=bqk_d[h, 1, :])
            b1e_sb = wqkp.tile([P, 1], f32, name="b1e_sb", tag="b1e")
            nc.sync.dma_start(out=b1e_sb, in_=b1e_d[h, :])
            gam_sb = wqkp.tile([P, 1], f32, name="gam_sb", tag="gam")
            gam_ap = gam_d[h : h + 1]
            nc.gpsimd.dma_start(
                out=gam_sb,
                in_=bass.AP(tensor=gam_ap.tensor, offset=gam_ap.offset, ap=[[0, P], [1, 1]]),
            )
            # residual rows h, h+8, ..., h+8*127 of x
            xres_sb = wqkp.tile([P, T], f32, name="xres_sb", tag="xres")
            x_all = x_d[:]
            nc.sync.dma_start(
                out=xres_sb,
                in_=bass.AP(tensor=x_all.tensor, offset=h * T, ap=[[H * T, P], [1, T]]),
            )
            q_sb = qkp.tile([P, T], ldt, name="q_sb", tag="q")
            k_sb = qkp.tile([P, T], ldt, name="k_sb", tag="k")
            out_sb = outp.tile([P, T], f32, name="out_sb", tag="ob")
            # K then Q, both chunks each; bias-adds hide under later MM groups
            for t2 in range(NT):
                tsl = slice(t2 * TCW, (t2 + 1) * TCW)
                ps_k = psB.tile([P, TCW], f32, name="ps_k", tag="qk", bufs=2)
                for ci in range(CT):
                    mm(ps_k, wqk_sb[:, ci, P : 2 * P], xl_sb[:, ci, tsl], ci == 0, ci == CT - 1)
                nc.vector.tensor_scalar_add(out=k_sb[:, tsl], in0=ps_k, scalar1=bk_sb)
            for t2 in range(NT):
                tsl = slice(t2 * TCW, (t2 + 1) * TCW)
                ps_q = psB.tile([P, TCW], f32, name="ps_q", tag="qk", bufs=2)
                for ci in range(CT):
                    mm(ps_q, wqk_sb[:, ci, 0:P], xl_sb[:, ci, tsl], ci == 0, ci == CT - 1)
                nc.vector.tensor_scalar_add(out=q_sb[:, tsl], in0=ps_q, scalar1=bq_sb)
            head_state[h] = dict(
                q=q_sb, k=k_sb, gam=gam_sb, b1e=b1e_sb, xres=xres_sb, out=out_sb
            )

        def emit_s1_half(c, first):
            h, t2 = c
            hs = head_state[h]
            tsl = slice(t2 * TCW, (t2 + 1) * TCW)
            if first:
                et_sb = expp.tile([P, ST, TCW], mdt, name="et_sb", tag="exp")
                chunk_state[c] = dict(et=et_sb)
            else:
                et_sb = chunk_state[c]["et"]
            rng = range(0, ST // 2) if first else range(ST // 2, ST)
            for si in rng:
                ps_e = psA.tile([P, TCW], f32, name="ps_e", tag="acc")
                mm(ps_e, hs["k"][:, si * P : (si + 1) * P], hs["q"][:, tsl], True, True)
                nc.scalar.activation(out=et_sb[:, si, :], in_=ps_e, func=AF.Exp)

        def emit_s1(c):
            emit_s1_half(c, True)
            emit_s1_half(c, False)

        def emit_s2_mm(c):
            h, t2 = c
            cs = chunk_state[c]
            et_sb = cs["et"]
            ps_o = psB.tile([P, TCW], f32, name="ps_o", tag="oo")
            for si in range(ST):
                mm(
                    ps_o,
                    vw1t_sb[:, si, h * P : (h + 1) * P],
                    et_sb[:, si, :],
                    si == 0,
                    si == ST - 1,
                )
            cs["ps_o"] = ps_o

        def emit_s2_s3(c):
            h, t2 = c
            hs = head_state[h]
            cs = chunk_state[c]
            tsl = slice(t2 * TCW, (t2 + 1) * TCW)
            et_sb = cs["et"]
            ps_o = cs["ps_o"]
            # Z: tree-sum the 8 s-tiles on DVE (free-dim adds), then a single
            # ones-matmul for the partition reduction + broadcast.
            r1 = hbuf.tile([P, 4, TCW], mdt, name="r1", tag="r1")
            nc.vector.tensor_add(r1, et_sb[:, 0:4, :], et_sb[:, 4:8, :])
            r2 = hbuf.tile([P, 2, TCW], mdt, name="r2", tag="r2")
            nc.vector.tensor_add(r2, r1[:, 0:2, :], r1[:, 2:4, :])
            etsum = hbuf.tile([P, TCW], mdt, name="etsum", tag="etsum")
            nc.vector.tensor_add(etsum, r2[:, 0, :], r2[:, 1, :])
            ps_z = psB.tile([P, TCW], f32, name="ps_z", tag="zf")
            mm(ps_z, ones_sb, etsum, True, True)
            # fc1 = relu(gamma * oW1/Z + xW1 + b1eff), all on DVE
            izg = hbuf.tile([P, TCW], f32, name="izg", tag="izg")
            nc.vector.reciprocal_approx_fast(out=izg, in_=ps_z)
            t1 = hbuf.tile([P, TCW], f32, name="t1", tag="t1")
            nc.vector.scalar_tensor_tensor(
                out=t1, in0=ps_o, scalar=hs["gam"], in1=izg, op0=ALU.mult, op1=ALU.mult
            )
            t2t = hbuf.tile([P, TCW], f32, name="t2t", tag="t2t")
            nc.vector.scalar_tensor_tensor(
                out=t2t, in0=t1, scalar=hs["b1e"], in1=xw1_sb[:, tsl], op0=ALU.add, op1=ALU.add
            )
            fc1 = hbuf.tile([P, TCW], mdt, name="fc1", tag="fc1")
            nc.vector.tensor_scalar_max(out=fc1, in0=t2t, scalar1=0.0)
            cs["fc1"] = fc1

        def emit_s4_s5(c):
            h, t2 = c
            hs = head_state[h]
            cs = chunk_state[c]
            tsl = slice(t2 * TCW, (t2 + 1) * TCW)
            ps_f = psB.tile([P, TCW], f32, name="ps_f", tag="zf")
            mm(ps_f, w2t_sb[:], cs["fc1"], True, True)
            ot = hbuf.tile([P, TCW], f32, name="ot", tag="ot")
            nc.scalar.activation(out=ot, in_=ps_f, func=AF.Relu, bias=b2_sb)
            nc.vector.tensor_add(hs["out"][:, tsl], ot, hs["xres"][:, tsl])
            out_all = out_d[:]
            nc.sync.dma_start(
                out=bass.AP(
                    tensor=out_all.tensor,
                    offset=h * T + t2 * TCW,
                    ap=[[H * T, P], [1, TCW]],
                ),
                in_=hs["out"][:, tsl],
            )

        # ---- phase A compute: xW1 first (smallest DMA deps), head-0 QK
        # next (overlaps w1wv DMA), then vW1T.
        for t2 in range(NT):
            tsl = slice(t2 * TCW, (t2 + 1) * TCW)
            ps_x = psA.tile([P, TCW], f32, name="ps_x", tag="acc")
            for ci in range(CT):
                mm(ps_x, w1t_sb[:, ci, :], xm_sb[:, ci, tsl], ci == 0, ci == CT - 1)
            nc.vector.tensor_copy(out=xw1_sb[:, tsl], in_=ps_x)

        # vW1T = x.T @ W1WvT for all heads  (s on partitions, h*128+j free)
        for si in range(ST):
            for jh in range(2):
                jsl = slice(jh * 512, (jh + 1) * 512)
                ps_v = psA.tile([P, TCW], f32, name="ps_v", tag="acc")
                for ci in range(CT):
                    mm(
                        ps_v,
                        xm_sb[:, ci, si * P : (si + 1) * P],
                        w1wv_sb[:, ci, jsl],
                        ci == 0,
                        ci == CT - 1,
                    )
                nc.vector.tensor_copy(out=vw1t_sb[:, si, jsl], in_=ps_v)

        chunks = [(h, t2) for h in range(H) for t2 in range(NT)]
        for i, c in enumerate(chunks):
            if c[1] == 0:
                emit_head_setup(c[0])
            # interleave: first half of this chunk's eT, then the previous
            # chunk's oW1 matmuls (gives the exp chain time to recycle the
            # eT psum slots), then the second half, then the rest.
            emit_s1_half(c, True)
            if i >= 1:
                emit_s2_mm(chunks[i - 1])
            emit_s1_half(c, False)
            if i >= 1:
                emit_s2_s3(chunks[i - 1])
            if i >= 2:
                emit_s4_s5(chunks[i - 2])
        emit_s2_mm(chunks[-1])
        emit_s2_s3(chunks[-1])
        emit_s4_s5(chunks[-2])
        emit_s4_s5(chunks[-1])

    nc.compile()
    return nc


def _prepare_inputs(inputs, cfg=CONFIG):
    import ml_dtypes

    logit_name, mlp_name = cfg
    l4 = logit_name in ("float32", "float32r")
    m4 = mlp_name in ("float32", "float32r")
    np_dt = {"bfloat16": ml_dtypes.bfloat16, "float16": np.float16}

    x = np.ascontiguousarray(np.asarray(inputs["x"], dtype=np.float32))
    Wq = np.asarray(inputs["Wq"], dtype=np.float32)
    bq = np.asarray(inputs["bq"], dtype=np.float32)
    Wk = np.asarray(inputs["Wk"], dtype=np.float32)
    bk = np.asarray(inputs["bk"], dtype=np.float32)
    Wv = np.asarray(inputs["Wv"], dtype=np.float32)
    bv = np.asarray(inputs["bv"], dtype=np.float32)
    gamma = np.asarray(inputs["gamma"], dtype=np.float32)
    W1 = np.asarray(inputs["W1"], dtype=np.float32)
    b1 = np.asarray(inputs["b1"], dtype=np.float32)
    W2 = np.asarray(inputs["W2"], dtype=np.float32)
    b2 = np.asarray(inputs["b2"], dtype=np.float32)

    # wqk[h, cp, ci, 0:128] = Wq[h].T[ci*128+cp, :]; 128:256 for Wk
    wqk = np.empty((H, P, CT, 256), dtype=np.float32)
    for h in range(H):
        wqk[h, :, :, 0:P] = Wq[h].T.reshape(CT, P, P).transpose(1, 0, 2)
        wqk[h, :, :, P : 2 * P] = Wk[h].T.reshape(CT, P, P).transpose(1, 0, 2)

    # w1wv[cp, ci, h*128+j] = (W1 @ Wv[h]).T[ci*128+cp, j]
    w1wvT = np.concatenate([(W1 @ Wv[h]).T for h in range(H)], axis=1)  # (C, H*128)
    w1wv = np.ascontiguousarray(w1wvT.reshape(CT, P, H * P).transpose(1, 0, 2))

    w1t = np.ascontiguousarray(W1.T.reshape(CT, P, P).transpose(1, 0, 2))
    w2t = np.ascontiguousarray(W2.T)

    bqk = np.stack([bq, bk], axis=1)  # (H, 2, P)
    b1v = bv @ W1.T  # (H, P): b1v[h] = W1 @ bv[h]
    b1e = b1[None, :] + gamma[:, None] * b1v  # (H, P)

    def fam(arr, four_byte, name):
        arr = np.ascontiguousarray(arr, dtype=np.float32)
        return arr if four_byte else np.ascontiguousarray(arr.astype(np_dt[name]))

    shared = {
        "wqk": fam(wqk, l4, logit_name),
        "w1wv": fam(w1wv, m4, mlp_name),
        "w1t": fam(w1t, m4, mlp_name),
        "w2t": fam(w2t, m4, mlp_name),
        "ones": fam(np.ones((P, P), dtype=np.float32), m4, mlp_name),
        "bqk": np.ascontiguousarray(bqk),
        "b1e": np.ascontiguousarray(b1e),
        "b2": np.ascontiguousarray(b2),
        "gam": np.ascontiguousarray(gamma),
    }
    narrow_names = {n for n, four in ((logit_name, l4), (mlp_name, m4)) if not four}
    in_maps = []
    for b in range(B):
        m = dict(shared)
        m["x"] = np.ascontiguousarray(x[b])
        for n in narrow_names:
            m[f"x_{n}"] = np.ascontiguousarray(x[b].astype(np_dt[n]))
        in_maps.append(m)
    return in_maps


def kernel(**inputs):
    from concourse.bass_utils import run_bass_kernel_spmd

    if CONFIG not in _module_cache:
        _module_cache[CONFIG] = _build_module(CONFIG)
    nc = _module_cache[CONFIG]

    in_maps = _prepare_inputs(inputs, CONFIG)
    res = run_bass_kernel_spmd(nc, in_maps, core_ids=list(range(B)))
    out = np.stack([res.results[b]["out"] for b in range(B)], axis=0)
    return out.astype(np.float32)
